# revision 1
# baseline (speedup 1.0000x reference)
"""Trainium2 Bass kernel for a pre-LN transformer block (B=4, T=2048, D=1024,
H=16, HS=64, FF=4096, causal attention).

Sharding: data-parallel over batch pairs x 2-way tensor-parallel
(heads for attention, columns/rows for FFN) with a pair AllReduce after the
attention output projection and after FC2 (Megatron style).

Core c (0..7): batch b = c//2, TP half = c%2 (8 local heads, 2048 local FF).
All activations live feature-major on chip (d on partitions, t on free dim);
the host transposes x in and the output back.
"""

import numpy as np
import ml_dtypes

import concourse.bacc as bacc
import concourse.bass as bass
import concourse.mybir as mybir
import concourse.tile as tile
from concourse.bass_utils import run_bass_kernel_spmd

BF16NP = ml_dtypes.bfloat16

B, T, D, H, HS, FF = 4, 2048, 1024, 16, 64, 4096
EPS = 1e-5
NCORES = 8
TP = 2
LH = H // TP          # 8 local heads
LHE = LH * HS         # 512 local head-embed width
LFF = FF // TP        # 2048 local FF
KD = D // 128         # 8 d k-tiles
KHE = LHE // 128      # 4 he k-tiles
KFF = LFF // 128      # 16 ff k-tiles
NCH = T // 512        # 4 t-chunks of 512
NST = T // 128        # 16 s-tiles of 128
PAIRS = [[0, 1], [2, 3], [4, 5], [6, 7]]

F32 = mybir.dt.float32
BF = mybir.dt.bfloat16


def _emit(nc, tc, t):
    mm = nc.tensor.matmul
    Alu = mybir.AluOpType
    Act = mybir.ActivationFunctionType

    xT_v = t["xT"].rearrange("(k p) t -> p k t", p=128)
    w1_v = t["w1"].rearrange("(k p) e -> p k e", p=128)
    w2_v = t["w2"].rearrange("(k p) e -> p k e", p=128)
    outT_v = t["outT"]

    # ---------------- persistent pools (LIFO stack bottom) ----------------
    dram = tc.alloc_tile_pool(name="dram", bufs=1, space="DRAM")
    ar1_in = [dram.tile([D, 512], F32, name=f"ar1i{c}") for c in range(NCH)]
    ar1_out = [dram.tile([D, 512], F32, name=f"ar1o{c}") for c in range(NCH)]
    ar2_in = [dram.tile([D, 512], F32, name=f"ar2i{c}") for c in range(NCH)]
    ar2_out = [dram.tile([D, 512], F32, name=f"ar2o{c}") for c in range(NCH)]

    consts = tc.alloc_tile_pool(name="consts", bufs=1)
    ones_col = consts.tile([128, 1], BF)        # lhsT for column-sum matmuls
    nc.vector.memset(ones_col, 1.0)

    # per-d-row parameter vectors, [128, KD] layout: [p, k] = v[k*128+p]
    g1_sb = consts.tile([128, KD], F32)
    be1_sb = consts.tile([128, KD], F32)
    g2_sb = consts.tile([128, KD], F32)
    be2_sb = consts.tile([128, KD], F32)
    bo_sb = consts.tile([128, KD], F32)
    b2_sb = consts.tile([128, KD], F32)
    b1_sb = consts.tile([128, KFF], F32)
    for name, dst in (("g1", g1_sb), ("be1", be1_sb), ("g2", g2_sb),
                      ("be2", be2_sb), ("bo", bo_sb), ("b2", b2_sb)):
        nc.sync.dma_start(out=dst, in_=t[name].rearrange("(k p) -> p k", p=128))
    nc.sync.dma_start(out=b1_sb, in_=t["b1l"].rearrange("(k p) -> p k", p=128))

    # causal masks for the 4 diagonal offsets: 1 where t_rel-s_rel-128*m>=0
    masks = []
    for midx in range(4):
        mk = consts.tile([128, 512], BF, name=f"mask{midx}")
        nc.vector.memset(mk, 1.0)
        nc.gpsimd.affine_select(
            out=mk, in_=mk, compare_op=Alu.is_ge, fill=0.0,
            base=-(midx * 128), channel_multiplier=-1, pattern=[[1, 512]])
        masks.append(mk)

    # wo + oT live until stage D
    wlate = tc.alloc_tile_pool(name="wlate", bufs=1)
    wo_sb = wlate.tile([128, KHE, D], BF, tag="wo")
    nc.sync.dma_start(out=wo_sb, in_=t["wo"].rearrange("(k p) e -> p k e", p=128))
    oT = [wlate.tile([128, KHE, 512], BF, name=f"oT{c}") for c in range(NCH)]

    # ========== Stages A+B+C merged, software-pipelined per chunk ==========
    with tc.tile_pool(name="abc", bufs=1) as ab:
        hT = [ab.tile([128, KD, 512], BF, name=f"hT{c}") for c in range(NCH)]
        kT = ab.tile([128, LH // 2, T], BF, tag="kT")
        qT = ab.tile([128, LH // 2, T], BF, tag="qT")
        # v rows with an interleaved ones column per head: [s, 8*(64+1)]
        vS = ab.tile([128, NST, LH * 65], BF, tag="vS")
        nc.vector.memset(vS, 1.0)
        wq_sb = ab.tile([128, KD, LHE], BF, tag="wq")
        wk_sb = ab.tile([128, KD, LHE], BF, tag="wk")
        wv_sb = ab.tile([128, KD, LHE], BF, tag="wv")
        for src, dst in ((t["wq"], wq_sb), (t["wk"], wk_sb), (t["wv"], wv_sb)):
            nc.sync.dma_start(out=dst,
                              in_=src.rearrange("(k p) e -> p k e", p=128))

        pools = {}

        def ln1(ci):
            c0 = ci * 512
            xf = [ab.tile([128, 512], F32, tag="xf", bufs=9, name="xf")
                  for _ in range(KD)]
            for k in range(KD):
                nc.sync.dma_start(out=xf[k], in_=xT_v[:, k, c0:c0 + 512])
            Ab, Bb = _ln_stats(nc, tc, ab, pools["pstat"], xf, ones_col, F32)
            for k in range(KD):
                _ln_apply(nc, ab, xf[k], Ab, Bb, g1_sb, be1_sb, k,
                          hT[ci][:, k, :], F32)

        def proj(ci):
            c0 = ci * 512
            # k / q projections for this chunk
            for w_sb, dst in ((wk_sb, kT), (wq_sb, qT)):
                for et in range(LH // 2):
                    ps = pools["pproj"].tile([128, 512], F32, tag="ps_proj", bufs=2,
                                    name="ps_proj")
                    for k in range(KD):
                        mm(out=ps, lhsT=w_sb[:, k, et * 128:(et + 1) * 128],
                           rhs=hT[ci][:, k, :],
                           start=(k == 0), stop=(k == KD - 1))
                    nc.vector.tensor_copy(
                        out=dst[:, et, c0:c0 + 512], in_=ps)
            # v projection (row-major, into the 65-strided layout)
            for sti in range(4):
                st = ci * 4 + sti
                ps = pools["pproj"].tile([128, LHE], F32, tag="ps_proj", bufs=2,
                                name="ps_v")
                for k in range(KD):
                    mm(out=ps, lhsT=hT[ci][:, k, sti * 128:sti * 128 + 128],
                       rhs=wv_sb[:, k, :],
                       start=(k == 0), stop=(k == KD - 1))
                nc.vector.tensor_copy(
                    out=vS[:, st, :].rearrange("p (h e) -> p h e",
                                               h=LH)[:, :, 0:64],
                    in_=ps.rearrange("p (h e) -> p h e", e=64))

        def attention(ci):
            c0 = ci * 512
            nb = 4 * (ci + 1)
            dn8 = ab.tile([LH, 512], F32, tag="dn8", bufs=1, name="dn8")
            o_us = {}
            for hp in range(LH // 2):
                po = [pools["poa"].tile([65, 512], F32, tag="po", bufs=2, name="po")
                      for _ in range(2)]
                for sb in range(nb):
                    s0 = sb * 128
                    pss = [pools["psc"].tile([128, 512], F32, tag="ps_sc", bufs=4,
                                    name="ps_sc") for _ in range(2)]
                    for hi in range(2):
                        e0 = hi * 64
                        mm(out=pss[hi],
                           lhsT=kT[e0:e0 + 64, hp, s0:s0 + 128],
                           rhs=qT[e0:e0 + 64, hp, c0:c0 + 512],
                           start=True, stop=True)
                    exs = []
                    for hi in range(2):
                        ex = ab.tile([128, 512], BF, tag="ex", bufs=3,
                                     name="ex")
                        nc.scalar.activation(out=ex, in_=pss[hi],
                                             func=Act.Exp,
                                             scale=float(HS) ** -0.5)
                        exs.append(ex)
                    midx = sb - 4 * ci
                    if midx >= 0:
                        for hi in range(2):
                            nc.vector.tensor_mul(out=exs[hi], in0=exs[hi],
                                                 in1=masks[midx])
                    for hi in range(2):
                        h_loc = hp * 2 + hi
                        mm(out=po[hi],
                           lhsT=vS[:, sb, h_loc * 65:h_loc * 65 + 65],
                           rhs=exs[hi],
                           start=(sb == 0), stop=(sb == nb - 1))
                for hi in range(2):
                    h_loc = hp * 2 + hi
                    # unnormalized o (bf16) kept until batched reciprocal
                    ou = ab.tile([64, 512], BF, tag="ou", bufs=8,
                                 name="ou")
                    o_us[h_loc] = ou
                    nc.vector.tensor_copy(out=ou, in_=po[hi][0:64, :])
                    dnr = ab.tile([1, 512], F32, tag="dnr", bufs=1,
                                  name="dnr")
                    nc.vector.tensor_copy(out=dnr, in_=po[hi][64:65, :])
                    nc.sync.dma_start(out=dn8[h_loc:h_loc + 1, :], in_=dnr)
            rec8 = ab.tile([LH, 512], F32, tag="rec8", bufs=1, name="rec8")
            nc.vector.reciprocal(out=rec8, in_=dn8)
            rb8 = ab.tile([LH, 512], BF, tag="rb8", bufs=2, name="rb8")
            nc.vector.tensor_copy(out=rb8, in_=rec8)
            for h_loc in range(LH):
                rbt = ab.tile([1, 512], BF, tag="rbt", bufs=1, name="rbt")
                nc.sync.dma_start(out=rbt, in_=rb8[h_loc:h_loc + 1, :])
                bc = ab.tile([64, 512], BF, tag="bc", bufs=1, name="bc")
                nc.gpsimd.partition_broadcast(bc, rbt)
                nc.vector.tensor_mul(
                    out=oT[ci][(h_loc % 2) * 64:(h_loc % 2) * 64 + 64,
                               h_loc // 2, :],
                    in0=o_us[h_loc], in1=bc)

        def wo_ar1(ci):
            for dt in range(KD):
                ps = pools["pproj"].tile([128, 512], F32, tag="ps_proj",
                                         bufs=2, name="ps_wo")
                for k in range(KHE):
                    mm(out=ps, lhsT=wo_sb[:, k, dt * 128:(dt + 1) * 128],
                       rhs=oT[ci][:, k, :],
                       start=(k == 0), stop=(k == KHE - 1))
                stg = ab.tile([128, 512], F32, tag="stg1", bufs=2,
                              name="stg1")
                nc.scalar.copy(out=stg, in_=ps)
                nc.sync.dma_start(
                    out=ar1_in[ci][dt * 128:(dt + 1) * 128, :], in_=stg)
            nc.gpsimd.collective_compute(
                "AllReduce", Alu.add, replica_groups=PAIRS,
                ins=[ar1_in[ci].opt()], outs=[ar1_out[ci].opt()])

        # LN1 for all chunks first (frees the stats psum banks), then
        # software pipeline: projections of chunk ci overlap attention ci-1
        with tc.tile_pool(name="statpsum", bufs=1, space="PSUM") as pstat_:
            pools["pstat"] = pstat_
            for ci in range(NCH):
                ln1(ci)
        with tc.tile_pool(name="projpsum", bufs=2, space="PSUM") as pproj_, \
             tc.tile_pool(name="scpsum", bufs=4, space="PSUM") as psc_, \
             tc.tile_pool(name="oaccpsum", bufs=2, space="PSUM") as poa_:
            pools["pproj"] = pproj_
            pools["psc"] = psc_
            pools["poa"] = poa_
            for ci in range(NCH + 1):
                if ci < NCH:
                    proj(ci)
                if ci >= 1:
                    attention(ci - 1)
                    wo_ar1(ci - 1)

    # ========== Stage D1: Wo partials + AllReduce for all chunks ==========
    with tc.tile_pool(name="de", bufs=1) as de, \
         tc.tile_pool(name="ln2psum", bufs=1, space="PSUM") as pstat2, \
         tc.tile_pool(name="upsum", bufs=3, space="PSUM") as pu, \
         tc.tile_pool(name="fpsum", bufs=3, space="PSUM") as pf:

        # ========== Stage D2+E per chunk ==========
        for ci in range(NCH):
            c0 = ci * 512
            # residual 1: xmid = x + attn + bo  (bf16 residual stream)
            xmid = de.tile([128, KD, 512], BF, tag="xmid", bufs=2,
                           name="xmid")
            ar1v = ar1_out[ci].rearrange("(k p) t -> p k t", p=128)
            for k in range(KD):
                ar_sb = de.tile([128, 512], F32, tag="ar1sb", bufs=2,
                                name="ar1sb")
                nc.sync.dma_start(out=ar_sb, in_=ar1v[:, k, :])
                xf2 = de.tile([128, 512], F32, tag="xf2", bufs=2, name="xf2")
                nc.sync.dma_start(out=xf2, in_=xT_v[:, k, c0:c0 + 512])
                nc.vector.scalar_tensor_tensor(
                    out=xmid[:, k, :], in0=ar_sb,
                    scalar=bo_sb[:, k:k + 1], in1=xf2,
                    op0=Alu.add, op1=Alu.add)

            # LN2 (xmid already bf16: feed matmuls directly)
            src = [xmid[:, k, :] for k in range(KD)]
            Ab2, Bb2 = _ln_stats(nc, tc, de, pstat2, src, ones_col, BF)
            h2 = de.tile([128, KD, 512], BF, tag="h2", bufs=2, name="h2")
            for k in range(KD):
                _ln_apply(nc, de, src[k], Ab2, Bb2, g2_sb, be2_sb, k,
                          h2[:, k, :], BF)

            # FFN up: u = relu(h2 @ W1 + b1)
            u = de.tile([128, KFF, 512], BF, tag="u", bufs=2, name="u")
            for fp in range(KFF // 2):
                w1t = de.tile([128, KD, 256], BF, tag="w1t", bufs=2,
                              name="w1t")
                nc.sync.dma_start(out=w1t,
                                  in_=w1_v[:, :, fp * 256:(fp + 1) * 256])
                for half in range(2):
                    fft = fp * 2 + half
                    ps = pu.tile([128, 512], F32, tag="ps_u", bufs=3,
                                 name="ps_u")
                    for k in range(KD):
                        mm(out=ps,
                           lhsT=w1t[:, k, half * 128:half * 128 + 128],
                           rhs=h2[:, k, :],
                           start=(k == 0), stop=(k == KD - 1))
                    nc.scalar.activation(out=u[:, fft, :], in_=ps,
                                         func=Act.Relu,
                                         bias=b1_sb[:, fft:fft + 1])
            # FFN down partial -> AllReduce
            w2a = de.tile([128, KFF // 2, D], BF, tag="w2t", bufs=2,
                          name="w2a")
            w2b = de.tile([128, KFF // 2, D], BF, tag="w2t", bufs=2,
                          name="w2b")
            nc.sync.dma_start(out=w2a, in_=w2_v[:, 0:KFF // 2, :])
            nc.sync.dma_start(out=w2b, in_=w2_v[:, KFF // 2:KFF, :])
            for dt in range(KD):
                ps = pf.tile([128, 512], F32, tag="ps_f", bufs=3,
                             name="ps_f")
                for k2 in range(KFF):
                    wt = w2a if k2 < KFF // 2 else w2b
                    mm(out=ps,
                       lhsT=wt[:, k2 % (KFF // 2),
                               dt * 128:(dt + 1) * 128],
                       rhs=u[:, k2, :],
                       start=(k2 == 0), stop=(k2 == KFF - 1))
                stg = de.tile([128, 512], F32, tag="stg2", bufs=3,
                              name="stg2")
                nc.scalar.copy(out=stg, in_=ps)
                nc.sync.dma_start(
                    out=ar2_in[ci][dt * 128:(dt + 1) * 128, :], in_=stg)
            nc.gpsimd.collective_compute(
                "AllReduce", Alu.add, replica_groups=PAIRS,
                ins=[ar2_in[ci].opt()], outs=[ar2_out[ci].opt()])
            # residual 2 + store
            ar2v = ar2_out[ci].rearrange("(k p) t -> p k t", p=128)
            for dt in range(KD):
                ar2_sb = de.tile([128, 512], F32, tag="ar2sb", bufs=2,
                                 name="ar2sb")
                nc.sync.dma_start(out=ar2_sb, in_=ar2v[:, dt, :])
                o_f = de.tile([128, 512], F32, tag="o_f", bufs=2, name="o_f")
                nc.vector.scalar_tensor_tensor(
                    out=o_f, in0=ar2_sb,
                    scalar=b2_sb[:, dt:dt + 1], in1=xmid[:, dt, :],
                    op0=Alu.add, op1=Alu.add)
                nc.sync.dma_start(
                    out=outT_v[dt * 128:(dt + 1) * 128, c0:c0 + 512],
                    in_=o_f)

    # release persistent pools in reverse stack order
    wlate.release()
    consts.release()
    dram.release()


def _ln_stats(nc, tc, pool, pstat, src_tiles, ones_col, in_dt):
    """src_tiles: list of KD [128,512] APs (f32 or bf16) for one t-chunk.
    Returns (Ab, Bb) broadcast tiles (dtype in_dt) so that
    normalized = src*Ab + Bb (gamma/beta applied separately)."""
    mm = nc.tensor.matmul
    Alu = mybir.AluOpType
    Act = mybir.ActivationFunctionType
    ps_s = pstat.tile([1, 512], F32, tag="ps_s", name="ps_s")
    ps_q = pstat.tile([1, 512], F32, tag="ps_q", name="ps_q")
    for k in range(len(src_tiles)):
        sq = pool.tile([128, 512], BF, tag="ln_sq", bufs=2, name="ln_sq")
        nc.scalar.activation(out=sq, in_=src_tiles[k], func=Act.Square)
        if in_dt == BF:
            xb = src_tiles[k]
        else:
            xbt = pool.tile([128, 512], BF, tag="ln_xb", bufs=2, name="ln_xb")
            nc.scalar.copy(out=xbt, in_=src_tiles[k])
            xb = xbt
        mm(out=ps_s, lhsT=ones_col, rhs=xb,
           start=(k == 0), stop=(k == KD - 1))
        mm(out=ps_q, lhsT=ones_col, rhs=sq,
           start=(k == 0), stop=(k == KD - 1))
    m_sb = pool.tile([1, 512], F32, tag="ln_m", bufs=1, name="ln_m")
    e2 = pool.tile([1, 512], F32, tag="ln_e2", bufs=1, name="ln_e2")
    nc.vector.tensor_scalar_mul(out=m_sb, in0=ps_s, scalar1=1.0 / D)
    nc.vector.tensor_scalar_mul(out=e2, in0=ps_q, scalar1=1.0 / D)
    msq = pool.tile([1, 512], F32, tag="ln_msq", bufs=1, name="ln_msq")
    nc.vector.tensor_mul(out=msq, in0=m_sb, in1=m_sb)
    var = pool.tile([1, 512], F32, tag="ln_var", bufs=1, name="ln_var")
    nc.vector.scalar_tensor_tensor(
        out=var, in0=e2, scalar=EPS, in1=msq,
        op0=Alu.add, op1=Alu.subtract)
    sd = pool.tile([1, 512], F32, tag="ln_sd", bufs=1, name="ln_sd")
    nc.scalar.activation(out=sd, in_=var, func=Act.Sqrt)
    a_row = pool.tile([1, 512], F32, tag="ln_a", bufs=1, name="ln_a")
    nc.vector.reciprocal(out=a_row, in_=sd)
    b_row = pool.tile([1, 512], F32, tag="ln_b", bufs=1, name="ln_b")
    nc.vector.scalar_tensor_tensor(
        out=b_row, in0=m_sb, scalar=-1.0, in1=a_row,
        op0=Alu.mult, op1=Alu.mult)
    if in_dt == BF:
        ac = pool.tile([1, 512], BF, tag="ln_ac", bufs=1, name="ln_ac")
        bc_ = pool.tile([1, 512], BF, tag="ln_bc", bufs=1, name="ln_bc")
        nc.vector.tensor_copy(out=ac, in_=a_row)
        nc.vector.tensor_copy(out=bc_, in_=b_row)
        a_row, b_row = ac, bc_
    Ab = pool.tile([128, 512], in_dt, tag="ln_Ab", bufs=2, name="ln_Ab")
    Bb = pool.tile([128, 512], in_dt, tag="ln_Bb", bufs=2, name="ln_Bb")
    nc.gpsimd.partition_broadcast(Ab, a_row)
    nc.gpsimd.partition_broadcast(Bb, b_row)
    return Ab, Bb


def _ln_apply(nc, pool, src_k, Ab, Bb, g_sb, be_sb, k, out_slice, in_dt):
    """out = (src*Ab + Bb)*g[k] + be[k], bf16."""
    Alu = mybir.AluOpType
    t1 = pool.tile([128, 512], in_dt, tag="ln_t1", bufs=2, name="ln_t1")
    nc.vector.tensor_mul(out=t1, in0=src_k, in1=Ab)
    nc.vector.tensor_add(out=t1, in0=t1, in1=Bb)
    nc.vector.tensor_scalar(
        out=out_slice, in0=t1,
        scalar1=g_sb[:, k:k + 1], scalar2=be_sb[:, k:k + 1],
        op0=Alu.mult, op1=Alu.add)


def _build():
    nc = bacc.Bacc("TRN2", target_bir_lowering=False, debug=False,
                   num_devices=NCORES)

    tensors = {}
    tensors["xT"] = nc.dram_tensor("xT", [D, T], F32, kind="ExternalInput").ap()
    for name, shape, dt in (
        ("wq", [D, LHE], BF), ("wk", [D, LHE], BF), ("wv", [D, LHE], BF),
        ("wo", [LHE, D], BF), ("w1", [D, LFF], BF), ("w2", [LFF, D], BF),
        ("b1l", [LFF], F32), ("bo", [D], F32), ("b2", [D], F32),
        ("g1", [D], F32), ("be1", [D], F32), ("g2", [D], F32),
        ("be2", [D], F32),
    ):
        tensors[name] = nc.dram_tensor(name, shape, dt,
                                       kind="ExternalInput").ap()
    tensors["outT"] = nc.dram_tensor("out", [D, T], F32,
                                     kind="ExternalOutput").ap()

    with tile.TileContext(nc, num_cores=NCORES) as tc:
        _emit(nc, tc, tensors)

    nc.compile()
    return nc


_NC_CACHE = None


def _get_nc():
    global _NC_CACHE
    if _NC_CACHE is None:
        _NC_CACHE = _build()
    return _NC_CACHE


def _shard_inputs(x, Wq, Wk, Wv, Wo, bo, W1, b1, W2, b2, g1, be1, g2, be2):
    """Build the 8 per-core input maps."""
    bf = lambda a: np.ascontiguousarray(a).astype(BF16NP)
    f32 = lambda a: np.ascontiguousarray(a, dtype=np.float32)

    in_maps = []
    for c in range(NCORES):
        b, half = divmod(c, TP)
        heads = slice(half * LH, (half + 1) * LH)
        ffs = slice(half * LFF, (half + 1) * LFF)
        hes = slice(half * LHE, (half + 1) * LHE)
        # [H, D, HS] slice -> concat local heads along last dim -> [D, LHE]
        wq_l = np.concatenate(list(np.asarray(Wq)[heads]), axis=1)
        wk_l = np.concatenate(list(np.asarray(Wk)[heads]), axis=1)
        wv_l = np.concatenate(list(np.asarray(Wv)[heads]), axis=1)
        in_maps.append({
            "xT": f32(np.asarray(x)[b].T),
            "wq": bf(wq_l), "wk": bf(wk_l), "wv": bf(wv_l),
            "wo": bf(np.asarray(Wo)[hes, :]),
            "w1": bf(np.asarray(W1)[:, ffs]), "w2": bf(np.asarray(W2)[ffs, :]),
            "b1l": f32(np.asarray(b1)[ffs]),
            "bo": f32(bo), "b2": f32(b2),
            "g1": f32(g1), "be1": f32(be1), "g2": f32(g2), "be2": f32(be2),
        })
    return in_maps


def kernel(x, Wq, Wk, Wv, Wo, bo, W1, b1, W2, b2, g1, be1, g2, be2,
           _trace=False):
    nc = _get_nc()
    in_maps = _shard_inputs(x, Wq, Wk, Wv, Wo, bo, W1, b1, W2, b2,
                            g1, be1, g2, be2)
    res = run_bass_kernel_spmd(nc, in_maps, list(range(NCORES)),
                               trace=_trace)
    out = np.empty((B, T, D), dtype=np.float32)
    for b in range(B):
        out[b] = res.results[TP * b]["out"].T
    if _trace:
        kernel.last_exec_time_ns = res.exec_time_ns
        kernel.last_results = res
    return out



# revision 8
# speedup vs baseline: 1.0589x; 1.0589x over previous
"""Trainium2 Bass kernel for a pre-LN transformer block (B=4, T=2048, D=1024,
H=16, HS=64, FF=4096, causal attention).

Sharding: data-parallel over batch pairs x 2-way tensor-parallel
(heads for attention, columns/rows for FFN) with a pair AllReduce (bf16)
after the attention output projection and after FC2 (Megatron style).

Core c (0..7): batch b = c//2, TP half = c%2 (8 local heads, 2048 local FF).
All activations live feature-major on chip (d on partitions, t on free dim);
the host transposes x in and the output back.

Schedule: 4 chunk-slots of 512 tokens, software-pipelined.  Slot ci emits
LN1(ci) -> QKV(ci) -> attention(ci) with FFN(ci-1) matmul thunks injected
between score/AV groups (keeps the PE dense so it stays at max p-state and
hides both AllReduces), then Wo(ci)+AR1(ci), AR2(ci-1), residual2(ci-2).
exp() batches two score blocks per activation to amortize ACT overhead.
"""

import numpy as np
import ml_dtypes

import concourse.bacc as bacc
import concourse.bass as bass
import concourse.mybir as mybir
import concourse.tile as tile
from concourse.bass_utils import run_bass_kernel_spmd

BF16NP = ml_dtypes.bfloat16

B, T, D, H, HS, FF = 4, 2048, 1024, 16, 64, 4096
EPS = 1e-5
NCORES = 8
TP = 2
LH = H // TP          # 8 local heads
LHE = LH * HS         # 512 local head-embed width
LFF = FF // TP        # 2048 local FF
KD = D // 128         # 8 d k-tiles
KHE = LHE // 128      # 4 he k-tiles
KFF = LFF // 128      # 16 ff k-tiles
NCH = T // 512        # 4 t-chunks of 512
NST = T // 128        # 16 s-tiles of 128
PAIRS = [[0, 1], [2, 3], [4, 5], [6, 7]]

F32 = mybir.dt.float32
BF = mybir.dt.bfloat16


def _emit(nc, tc, t):
    mm = nc.tensor.matmul
    Alu = mybir.AluOpType
    Act = mybir.ActivationFunctionType

    xT_v = t["xT"].rearrange("(k p) t -> p k t", p=128)
    w1_v = t["w1"].rearrange("(k p) e -> p k e", p=128)
    w2_v = t["w2"].rearrange("(k p) e -> p k e", p=128)
    outT_v = t["outT"]

    # ---------------- persistent pools ----------------
    dram = tc.alloc_tile_pool(name="dram", bufs=1, space="DRAM")
    ar1_in = [dram.tile([D, 512], BF, name=f"ar1i{c}") for c in range(NCH)]
    ar1_out = [dram.tile([D, 512], BF, name=f"ar1o{c}") for c in range(NCH)]
    ar2_in = [dram.tile([D, 512], BF, name=f"ar2i{c}") for c in range(NCH)]
    ar2_out = [dram.tile([D, 512], BF, name=f"ar2o{c}") for c in range(NCH)]
    xmid_dram = dram.tile([D, T], BF, name="xmidd")
    xmid_v = xmid_dram.rearrange("(k p) t -> p k t", p=128)

    consts = tc.alloc_tile_pool(name="consts", bufs=1)
    ones_col = consts.tile([128, 1], BF)
    nc.vector.memset(ones_col, 1.0)

    g1_sb = consts.tile([128, KD], F32)
    be1_sb = consts.tile([128, KD], F32)
    g2_sb = consts.tile([128, KD], F32)
    be2_sb = consts.tile([128, KD], F32)
    bo_sb = consts.tile([128, KD], F32)
    b2_sb = consts.tile([128, KD], F32)
    b1_sb = consts.tile([128, KFF], F32)
    for name, dst in (("g1", g1_sb), ("be1", be1_sb), ("g2", g2_sb),
                      ("be2", be2_sb), ("bo", bo_sb), ("b2", b2_sb)):
        nc.sync.dma_start(out=dst, in_=t[name].rearrange("(k p) -> p k", p=128))
    nc.sync.dma_start(out=b1_sb, in_=t["b1l"].rearrange("(k p) -> p k", p=128))

    masks = []
    for midx in range(4):
        mk = consts.tile([128, 512], BF, name=f"mask{midx}")
        nc.vector.memset(mk, 1.0)
        nc.gpsimd.affine_select(
            out=mk, in_=mk, compare_op=Alu.is_ge, fill=0.0,
            base=-(midx * 128), channel_multiplier=-1, pattern=[[1, 512]])
        masks.append(mk)

    # resident weights: QKV + Wo
    wres = tc.alloc_tile_pool(name="wres", bufs=1)
    wq_sb = wres.tile([128, KD, LHE], BF, tag="wq")
    wk_sb = wres.tile([128, KD, LHE], BF, tag="wk")
    wv_sb = wres.tile([128, KD, LHE], BF, tag="wv")
    wo_sb = wres.tile([128, KHE, D], BF, tag="wo")
    for src, dst in ((t["wq"], wq_sb), (t["wk"], wk_sb), (t["wv"], wv_sb)):
        nc.sync.dma_start(out=dst, in_=src.rearrange("(k p) e -> p k e", p=128))
    nc.sync.dma_start(out=wo_sb, in_=t["wo"].rearrange("(k p) e -> p k e", p=128))

    # persistent attention state: K and V for the full sequence
    attp = tc.alloc_tile_pool(name="attp", bufs=1)
    kT = attp.tile([128, LH // 2, T], BF, tag="kT")
    vS = attp.tile([128, NST, LH * 65], BF, tag="vS")
    nc.vector.memset(vS, 1.0)

    with tc.tile_pool(name="work", bufs=1) as wk_pool, \
         tc.tile_pool(name="wstream", bufs=1) as wstr, \
         tc.tile_pool(name="scpsum", bufs=2, space="PSUM") as psc, \
         tc.tile_pool(name="oapsum", bufs=2, space="PSUM") as poa, \
         tc.tile_pool(name="genpsum", bufs=2, space="PSUM") as pgen:

        wp = wk_pool
        xb_tiles = {}
        hT_tiles = {}
        h2_tiles = {}
        u_tiles = {}
        qT_tiles = {}

        def gen_ps(tag="gen", shape=(128, 512)):
            return pgen.tile(list(shape), F32, tag="gen", bufs=2, name=tag)

        # ---------- LN helpers ----------
        def ln_stats(src_k, out_dt):
            """src_k: list of KD [128,512] bf16 APs. Returns (Ab, Bb) bf16."""
            ps_s = gen_ps("ps_s", (1, 512))
            ps_q = gen_ps("ps_q", (1, 512))
            for k in range(KD):
                sq = wp.tile([128, 512], BF, tag="ln_sq", bufs=2, name="ln_sq")
                nc.vector.tensor_mul(out=sq, in0=src_k[k], in1=src_k[k])
                mm(out=ps_s, lhsT=ones_col, rhs=src_k[k],
                   start=(k == 0), stop=(k == KD - 1))
                mm(out=ps_q, lhsT=ones_col, rhs=sq,
                   start=(k == 0), stop=(k == KD - 1))
            m_sb = wp.tile([1, 512], F32, tag="ln_m", bufs=1, name="ln_m")
            e2 = wp.tile([1, 512], F32, tag="ln_e2", bufs=1, name="ln_e2")
            nc.vector.tensor_scalar_mul(out=m_sb, in0=ps_s, scalar1=1.0 / D)
            nc.vector.tensor_scalar_mul(out=e2, in0=ps_q, scalar1=1.0 / D)
            msq = wp.tile([1, 512], F32, tag="ln_msq", bufs=1, name="ln_msq")
            nc.vector.tensor_mul(out=msq, in0=m_sb, in1=m_sb)
            var = wp.tile([1, 512], F32, tag="ln_var", bufs=1, name="ln_var")
            nc.vector.scalar_tensor_tensor(
                out=var, in0=e2, scalar=EPS, in1=msq,
                op0=Alu.add, op1=Alu.subtract)
            sd = wp.tile([1, 512], F32, tag="ln_sd", bufs=1, name="ln_sd")
            nc.scalar.activation(out=sd, in_=var, func=Act.Sqrt)
            a_row = wp.tile([1, 512], F32, tag="ln_a", bufs=1, name="ln_a")
            nc.vector.reciprocal(out=a_row, in_=sd)
            b_row = wp.tile([1, 512], F32, tag="ln_b", bufs=1, name="ln_b")
            nc.vector.scalar_tensor_tensor(
                out=b_row, in0=m_sb, scalar=-1.0, in1=a_row,
                op0=Alu.mult, op1=Alu.mult)
            ac = wp.tile([1, 512], BF, tag="ln_ac", bufs=1, name="ln_ac")
            bc_ = wp.tile([1, 512], BF, tag="ln_bc", bufs=1, name="ln_bc")
            nc.vector.tensor_copy(out=ac, in_=a_row)
            nc.vector.tensor_copy(out=bc_, in_=b_row)
            Ab = wp.tile([128, 512], BF, tag="ln_Ab", bufs=1, name="ln_Ab")
            Bb = wp.tile([128, 512], BF, tag="ln_Bb", bufs=1, name="ln_Bb")
            nc.gpsimd.partition_broadcast(Ab, ac)
            nc.gpsimd.partition_broadcast(Bb, bc_)
            return Ab, Bb

        def ln_apply(src, Ab, Bb, g_sb, be_sb, k, out_slice):
            t1 = wp.tile([128, 512], BF, tag="ln_t1", bufs=2, name="ln_t1")
            nc.vector.tensor_mul(out=t1, in0=src, in1=Ab)
            nc.vector.tensor_add(out=t1, in0=t1, in1=Bb)
            nc.vector.tensor_scalar(
                out=out_slice, in0=t1,
                scalar1=g_sb[:, k:k + 1], scalar2=be_sb[:, k:k + 1],
                op0=Alu.mult, op1=Alu.add)

        # ---------- stage functions ----------
        def ln1(ci):
            c0 = ci * 512
            xb = wp.tile([128, KD, 512], BF, tag="xb", bufs=1, name="xb")
            xb_tiles[ci] = xb
            for k in range(KD):
                xf = wp.tile([128, 512], F32, tag="xf", bufs=2, name="xf")
                nc.sync.dma_start(out=xf, in_=xT_v[:, k, c0:c0 + 512])
                nc.vector.tensor_copy(out=xb[:, k, :], in_=xf)
            src = [xb[:, k, :] for k in range(KD)]
            Ab, Bb = ln_stats(src, BF)
            hT = wp.tile([128, KD, 512], BF, tag="hT", bufs=1, name="hT")
            hT_tiles[ci] = hT
            for k in range(KD):
                ln_apply(src[k], Ab, Bb, g1_sb, be1_sb, k, hT[:, k, :])

        def qkv(ci):
            c0 = ci * 512
            hT = hT_tiles[ci]
            qT = wp.tile([128, LH // 2, 512], BF, tag="qT", bufs=1, name="qT")
            qT_tiles[ci] = qT
            for w_sb, dst, dslice in (
                    (wk_sb, kT, lambda et: kT[:, et, c0:c0 + 512]),
                    (wq_sb, qT, lambda et: qT[:, et, :])):
                for et in range(LH // 2):
                    ps = gen_ps("ps_proj")
                    for k in range(KD):
                        mm(out=ps, lhsT=w_sb[:, k, et * 128:(et + 1) * 128],
                           rhs=hT[:, k, :],
                           start=(k == 0), stop=(k == KD - 1))
                    nc.vector.tensor_copy(out=dslice(et), in_=ps)
            for sti in range(4):
                st = ci * 4 + sti
                ps = gen_ps("ps_v")
                for k in range(KD):
                    mm(out=ps, lhsT=hT[:, k, sti * 128:sti * 128 + 128],
                       rhs=wv_sb[:, k, :],
                       start=(k == 0), stop=(k == KD - 1))
                nc.vector.tensor_copy(
                    out=vS[:, st, :].rearrange("p (h e) -> p h e",
                                               h=LH)[:, :, 0:64],
                    in_=ps.rearrange("p (h e) -> p h e", e=64))

        def att_head(ci, h, dn8, ou_map, inject):
            """scores+exp+AV for one head; inject() emits FFN thunks."""
            nb = 4 * (ci + 1)
            ngrp = nb // 2
            hp, hi = h // 2, h % 2
            e0 = hi * 64
            qT = qT_tiles[ci]
            po = poa.tile([65, 512], F32, tag="po", bufs=2, name="po")
            for g in range(ngrp):
                grp = psc.tile([128, 2, 512], F32, tag="ps_sc", bufs=2,
                               name="ps_sc")
                for j in range(2):
                    sb = 2 * g + j
                    mm(out=grp[:, j, :],
                       lhsT=kT[e0:e0 + 64, hp, sb * 128:(sb + 1) * 128],
                       rhs=qT[e0:e0 + 64, hp, :],
                       start=True, stop=True)
                exg = wp.tile([128, 2, 512], BF, tag="ex", bufs=3, name="ex")
                nc.scalar.activation(out=exg, in_=grp, func=Act.Exp,
                                     scale=float(HS) ** -0.5)
                base = 4 * ci
                for j in range(2):
                    sb = 2 * g + j
                    if sb >= base:
                        nc.vector.tensor_mul(out=exg[:, j, :],
                                             in0=exg[:, j, :],
                                             in1=masks[sb - base])
                for j in range(2):
                    sb = 2 * g + j
                    mm(out=po, lhsT=vS[:, sb, h * 65:h * 65 + 65],
                       rhs=exg[:, j, :],
                       start=(sb == 0), stop=(sb == nb - 1))
                inject()
            ou = wp.tile([64, 512], BF, tag="ou", bufs=8, name="ou")
            ou_map[h] = ou
            nc.vector.tensor_copy(out=ou, in_=po[0:64, :])
            dnr = wp.tile([1, 512], F32, tag="dnr", bufs=2, name="dnr")
            nc.vector.tensor_copy(out=dnr, in_=po[64:65, :])
            nc.sync.dma_start(out=dn8[h:h + 1, :], in_=dnr)

        def att_norm(ci, dn8, ou_map, oT):
            rec8 = wp.tile([LH, 512], F32, tag="rec8", bufs=1, name="rec8")
            nc.vector.reciprocal(out=rec8, in_=dn8)
            rb8 = wp.tile([LH, 512], BF, tag="rb8", bufs=1, name="rb8")
            nc.vector.tensor_copy(out=rb8, in_=rec8)
            for h in range(LH):
                rbt = wp.tile([1, 512], BF, tag="rbt", bufs=2, name="rbt")
                nc.sync.dma_start(out=rbt, in_=rb8[h:h + 1, :])
                bc = wp.tile([64, 512], BF, tag="bc", bufs=2, name="bc")
                nc.gpsimd.partition_broadcast(bc, rbt)
                nc.vector.tensor_mul(
                    out=oT[(h % 2) * 64:(h % 2) * 64 + 64, h // 2, :],
                    in0=ou_map[h], in1=bc)

        def wo_ar1(ci, oT):
            for dt in range(KD):
                ps = gen_ps("ps_wo")
                for k in range(KHE):
                    mm(out=ps, lhsT=wo_sb[:, k, dt * 128:(dt + 1) * 128],
                       rhs=oT[:, k, :],
                       start=(k == 0), stop=(k == KHE - 1))
                stg = wp.tile([128, 512], BF, tag="stg", bufs=3, name="stg1")
                nc.vector.tensor_copy(out=stg, in_=ps)
                nc.sync.dma_start(
                    out=ar1_in[ci][dt * 128:(dt + 1) * 128, :], in_=stg)
            nc.gpsimd.collective_compute(
                "AllReduce", Alu.add, replica_groups=PAIRS,
                ins=[ar1_in[ci].opt()], outs=[ar1_out[ci].opt()])

        def res1_ln2(cj):
            c0 = cj * 512
            ar1v = ar1_out[cj].rearrange("(k p) t -> p k t", p=128)
            xmid = wp.tile([128, KD, 512], BF, tag="xmid", bufs=1, name="xmid")
            for k in range(KD):
                ar_sb = wp.tile([128, 512], BF, tag="arsb", bufs=3,
                                name="ar1sb")
                nc.sync.dma_start(out=ar_sb, in_=ar1v[:, k, :])
                xf2 = wp.tile([128, 512], F32, tag="xf", bufs=2, name="xf2")
                nc.sync.dma_start(out=xf2, in_=xT_v[:, k, c0:c0 + 512])
                nc.vector.scalar_tensor_tensor(
                    out=xmid[:, k, :], in0=ar_sb,
                    scalar=bo_sb[:, k:k + 1], in1=xf2,
                    op0=Alu.add, op1=Alu.add)
                nc.sync.dma_start(out=xmid_v[:, k, c0:c0 + 512],
                                  in_=xmid[:, k, :])
            src = [xmid[:, k, :] for k in range(KD)]
            Ab2, Bb2 = ln_stats(src, BF)
            h2 = wp.tile([128, KD, 512], BF, tag="h2", bufs=1, name="h2")
            h2_tiles[cj] = h2
            for k in range(KD):
                ln_apply(src[k], Ab2, Bb2, g2_sb, be2_sb, k, h2[:, k, :])

        def ffn_thunks(cj):
            """Return a list of thunks emitting FFN(cj) in small tensor bites.
            Must be called after res1_ln2(cj).  W1/W2 piece DMAs are issued
            by the first few thunks (bufs rotation auto-prefetches)."""
            h2 = h2_tiles.pop(cj)
            u = wp.tile([128, KFF, 512], BF, tag="u", bufs=1, name="u")
            thunks = []
            w1p_tiles = {}
            w2p_tiles = {}
            ps_box = {}

            def load_w1p(p):
                def f():
                    w1p = wstr.tile([128, KD, 256], BF, tag="w1p", bufs=2,
                                    name="w1p")
                    w1p_tiles[p] = w1p
                    nc.sync.dma_start(out=w1p,
                                      in_=w1_v[:, :, p * 256:(p + 1) * 256])
                return f

            def load_w2p(p):
                def f():
                    w2p = wstr.tile([128, KFF, 128], BF, tag="w2p", bufs=2,
                                    name="w2p")
                    w2p_tiles[p] = w2p
                    nc.sync.dma_start(out=w2p,
                                      in_=w2_v[:, :, p * 128:(p + 1) * 128])
                return f

            def up_half(fft, half):
                def f():
                    if half == 0:
                        ps_box[("u", fft)] = gen_ps("ps_u")
                    ps = ps_box[("u", fft)]
                    w1p = w1p_tiles[fft // 2]
                    col = (fft % 2) * 128
                    for kk in range(4):
                        k = half * 4 + kk
                        mm(out=ps, lhsT=w1p[:, k, col:col + 128],
                           rhs=h2[:, k, :],
                           start=(k == 0), stop=(k == KD - 1))
                    if half == 1:
                        nc.scalar.activation(out=u[:, fft, :], in_=ps,
                                             func=Act.Relu,
                                             bias=b1_sb[:, fft:fft + 1])
                        del ps_box[("u", fft)]
                return f

            def down_half(dt, half):
                def f():
                    if half == 0:
                        ps_box[("d", dt)] = gen_ps("ps_f")
                    ps = ps_box[("d", dt)]
                    w2p = w2p_tiles[dt]
                    for kk in range(8):
                        k2 = half * 8 + kk
                        mm(out=ps, lhsT=w2p[:, k2, :],
                           rhs=u[:, k2, :],
                           start=(k2 == 0), stop=(k2 == KFF - 1))
                    if half == 1:
                        stg = wp.tile([128, 512], BF, tag="stg", bufs=3,
                                      name="stg2")
                        nc.vector.tensor_copy(out=stg, in_=ps)
                        nc.sync.dma_start(
                            out=ar2_in[cj][dt * 128:(dt + 1) * 128, :],
                            in_=stg)
                        del ps_box[("d", dt)]
                        if dt == KD - 1:
                            nc.gpsimd.collective_compute(
                                "AllReduce", Alu.add, replica_groups=PAIRS,
                                ins=[ar2_in[cj].opt()],
                                outs=[ar2_out[cj].opt()])
                return f

            # W1 pieces 0/1 load now (early in the slot, ahead of their
            # consumers); later pieces go through thunks — the pool's bufs
            # rotation turns each load into a one-ahead prefetch.
            load_w1p(0)()
            load_w1p(1)()
            for fft in range(KFF):
                if fft % 2 == 0 and 2 + fft // 2 < KFF // 2:
                    thunks.append(load_w1p(2 + fft // 2))
                if fft == KFF - 2:
                    thunks.append(load_w2p(0))
                    thunks.append(load_w2p(1))
                thunks.append(up_half(fft, 0))
                thunks.append(up_half(fft, 1))
            for dt in range(KD):
                if 2 + dt < KD:
                    thunks.append(load_w2p(2 + dt))
                thunks.append(down_half(dt, 0))
                thunks.append(down_half(dt, 1))
            return thunks

        def res2(cj):
            c0 = cj * 512
            ar2v = ar2_out[cj].rearrange("(k p) t -> p k t", p=128)
            for k in range(KD):
                a2 = wp.tile([128, 512], BF, tag="arsb", bufs=3, name="ar2sb")
                nc.sync.dma_start(out=a2, in_=ar2v[:, k, :])
                xm = wp.tile([128, 512], BF, tag="xm2", bufs=2, name="xm2")
                nc.sync.dma_start(out=xm, in_=xmid_v[:, k, c0:c0 + 512])
                o_f = wp.tile([128, 512], F32, tag="o_f", bufs=2, name="o_f")
                nc.vector.scalar_tensor_tensor(
                    out=o_f, in0=a2, scalar=b2_sb[:, k:k + 1], in1=xm,
                    op0=Alu.add, op1=Alu.add)
                nc.sync.dma_start(
                    out=outT_v[k * 128:(k + 1) * 128, c0:c0 + 512], in_=o_f)

        # ---------- main schedule ----------
        for ci in range(NCH):
            ln1(ci)
            qkv(ci)

            dn8 = wp.tile([LH, 512], F32, tag="dn8", bufs=1, name="dn8")
            ou_map = {}
            oT = wp.tile([128, KHE, 512], BF, tag="oT", bufs=1, name="oT")

            pending = []
            state = {"i": 0}

            def inject():
                if state["i"] < len(pending):
                    pending[state["i"]]()
                    state["i"] += 1

            noop = lambda: None
            # heads 0-2 run clean; res1+LN2(ci-1) after head 2, then FFN(ci-1)
            # thunks are injected after every score/AV group of heads 3-7.
            for h in range(3):
                att_head(ci, h, dn8, ou_map, noop)
                if h == 1 and ci >= 1:
                    res1_ln2(ci - 1)
                    pending.extend(ffn_thunks(ci - 1))
            ngrp_rem = (4 * (ci + 1) // 2) * (LH - 3)
            per = -(-len(pending) // max(ngrp_rem, 1))  # ceil

            def inject_n():
                for _ in range(per):
                    inject()

            for h in range(3, LH):
                att_head(ci, h, dn8, ou_map, inject_n)
            while state["i"] < len(pending):
                inject()

            att_norm(ci, dn8, ou_map, oT)
            wo_ar1(ci, oT)
            if ci >= 2:
                res2(ci - 2)

        # ---------- tail ----------
        res1_ln2(NCH - 1)
        for th in ffn_thunks(NCH - 1):
            th()
        res2(NCH - 2)
        res2(NCH - 1)

    attp.release()
    wres.release()
    consts.release()
    dram.release()


def _build():
    nc = bacc.Bacc("TRN2", target_bir_lowering=False, debug=False,
                   num_devices=NCORES)

    tensors = {}
    tensors["xT"] = nc.dram_tensor("xT", [D, T], F32, kind="ExternalInput").ap()
    for name, shape, dt in (
        ("wq", [D, LHE], BF), ("wk", [D, LHE], BF), ("wv", [D, LHE], BF),
        ("wo", [LHE, D], BF), ("w1", [D, LFF], BF), ("w2", [LFF, D], BF),
        ("b1l", [LFF], F32), ("bo", [D], F32), ("b2", [D], F32),
        ("g1", [D], F32), ("be1", [D], F32), ("g2", [D], F32),
        ("be2", [D], F32),
    ):
        tensors[name] = nc.dram_tensor(name, shape, dt,
                                       kind="ExternalInput").ap()
    tensors["outT"] = nc.dram_tensor("out", [D, T], F32,
                                     kind="ExternalOutput").ap()

    with tile.TileContext(nc, num_cores=NCORES) as tc:
        _emit(nc, tc, tensors)

    nc.compile()
    return nc


_NC_CACHE = None


def _get_nc():
    global _NC_CACHE
    if _NC_CACHE is None:
        _NC_CACHE = _build()
    return _NC_CACHE


def _shard_inputs(x, Wq, Wk, Wv, Wo, bo, W1, b1, W2, b2, g1, be1, g2, be2):
    """Build the 8 per-core input maps."""
    bf = lambda a: np.ascontiguousarray(a).astype(BF16NP)
    f32 = lambda a: np.ascontiguousarray(a, dtype=np.float32)

    in_maps = []
    for c in range(NCORES):
        b, half = divmod(c, TP)
        heads = slice(half * LH, (half + 1) * LH)
        ffs = slice(half * LFF, (half + 1) * LFF)
        hes = slice(half * LHE, (half + 1) * LHE)
        wq_l = np.concatenate(list(np.asarray(Wq)[heads]), axis=1)
        wk_l = np.concatenate(list(np.asarray(Wk)[heads]), axis=1)
        wv_l = np.concatenate(list(np.asarray(Wv)[heads]), axis=1)
        in_maps.append({
            "xT": f32(np.asarray(x)[b].T),
            "wq": bf(wq_l), "wk": bf(wk_l), "wv": bf(wv_l),
            "wo": bf(np.asarray(Wo)[hes, :]),
            "w1": bf(np.asarray(W1)[:, ffs]), "w2": bf(np.asarray(W2)[ffs, :]),
            "b1l": f32(np.asarray(b1)[ffs]),
            "bo": f32(bo), "b2": f32(b2),
            "g1": f32(g1), "be1": f32(be1), "g2": f32(g2), "be2": f32(be2),
        })
    return in_maps


def kernel(x, Wq, Wk, Wv, Wo, bo, W1, b1, W2, b2, g1, be1, g2, be2,
           _trace=False):
    nc = _get_nc()
    in_maps = _shard_inputs(x, Wq, Wk, Wv, Wo, bo, W1, b1, W2, b2,
                            g1, be1, g2, be2)
    res = run_bass_kernel_spmd(nc, in_maps, list(range(NCORES)),
                               trace=_trace)
    out = np.empty((B, T, D), dtype=np.float32)
    for b in range(B):
        out[b] = res.results[TP * b]["out"].T
    if _trace:
        kernel.last_exec_time_ns = res.exec_time_ns
        kernel.last_results = res
    return out


# revision 9
# speedup vs baseline: 1.2146x; 1.1470x over previous
"""Trainium2 Bass kernel for a pre-LN transformer block (B=4, T=2048, D=1024,
H=16, HS=64, FF=4096, causal attention).

Sharding: data-parallel over batch pairs x 2-way tensor-parallel
(heads for attention, columns/rows for FFN) with a pair AllReduce (bf16)
after the attention output projection and after FC2 (Megatron style).

Core c (0..7): batch b = c//2, TP half = c%2 (8 local heads, 2048 local FF).
Activations are feature-major on chip (d on partitions, t on free dim);
the host transposes x in (bf16) and the output back.

Schedule per 512-token chunk-slot ci (software-pipelined, depth 2-3):
  QKV(ci) -> attention(ci) -> Wo(ci)+AR1(ci) -> LN1(ci+1) -> FC2(ci-1)+AR2(ci-1)
  -> residual2(ci-2)
Attention interleaves the score matmuls of head h with the AV matmuls of
head h-1 so the PE never waits on exp() (which runs one head behind on ACT),
and FC1(ci-1) thunks are injected between score/AV pairs to absorb the
ACT-vs-PE rate gap.  residual1+LN2(ci-1) is emitted two heads into
attention(ci) so its DVE work never blocks and AR1 latency is hidden.
"""

import numpy as np
import ml_dtypes

import concourse.bacc as bacc
import concourse.bass as bass
import concourse.mybir as mybir
import concourse.tile as tile
from concourse.bass_utils import run_bass_kernel_spmd

BF16NP = ml_dtypes.bfloat16

B, T, D, H, HS, FF = 4, 2048, 1024, 16, 64, 4096
EPS = 1e-5
NCORES = 8
TP = 2
LH = H // TP          # 8 local heads
LHE = LH * HS         # 512 local head-embed width
LFF = FF // TP        # 2048 local FF
KD = D // 128         # 8 d k-tiles
KHE = LHE // 128      # 4 he k-tiles
KFF = LFF // 128      # 16 ff k-tiles
NCH = T // 512        # 4 t-chunks of 512
NST = T // 128        # 16 s-tiles of 128
PAIRS = [[0, 1], [2, 3], [4, 5], [6, 7]]

F32 = mybir.dt.float32
BF = mybir.dt.bfloat16


def _emit(nc, tc, t):
    mm = nc.tensor.matmul
    Alu = mybir.AluOpType
    Act = mybir.ActivationFunctionType

    xb_v = t["xbT"].rearrange("(k p) t -> p k t", p=128)
    w1_v = t["w1"].rearrange("(k p) e -> p k e", p=128)
    w2_v = t["w2"].rearrange("(k p) e -> p k e", p=128)
    outT_v = t["outT"]

    # ---------------- persistent pools ----------------
    dram = tc.alloc_tile_pool(name="dram", bufs=1, space="DRAM")
    ar1_in = [dram.tile([D, 512], BF, name=f"ar1i{c}") for c in range(NCH)]
    ar1_out = [dram.tile([D, 512], BF, name=f"ar1o{c}") for c in range(NCH)]
    ar2_in = [dram.tile([D, 512], BF, name=f"ar2i{c}") for c in range(NCH)]
    ar2_out = [dram.tile([D, 512], BF, name=f"ar2o{c}") for c in range(NCH)]
    xmid_dram = dram.tile([D, T], BF, name="xmidd")
    xmid_v = xmid_dram.rearrange("(k p) t -> p k t", p=128)

    consts = tc.alloc_tile_pool(name="consts", bufs=1)
    ones_col = consts.tile([128, 1], BF)
    nc.vector.memset(ones_col, 1.0)

    g1_sb = consts.tile([128, KD], F32)
    be1_sb = consts.tile([128, KD], F32)
    g2_sb = consts.tile([128, KD], F32)
    be2_sb = consts.tile([128, KD], F32)
    bo_sb = consts.tile([128, KD], F32)
    b2_sb = consts.tile([128, KD], F32)
    b1_sb = consts.tile([128, KFF], F32)
    for name, dst in (("g1", g1_sb), ("be1", be1_sb), ("g2", g2_sb),
                      ("be2", be2_sb), ("bo", bo_sb), ("b2", b2_sb)):
        nc.sync.dma_start(out=dst, in_=t[name].rearrange("(k p) -> p k", p=128))
    nc.sync.dma_start(out=b1_sb, in_=t["b1l"].rearrange("(k p) -> p k", p=128))

    masks = []
    for midx in range(4):
        mk = consts.tile([128, 512], BF, name=f"mask{midx}")
        nc.vector.memset(mk, 1.0)
        nc.gpsimd.affine_select(
            out=mk, in_=mk, compare_op=Alu.is_ge, fill=0.0,
            base=-(midx * 128), channel_multiplier=-1, pattern=[[1, 512]])
        masks.append(mk)

    # resident weights: QKV + Wo
    wres = tc.alloc_tile_pool(name="wres", bufs=1)
    wq_sb = wres.tile([128, KD, LHE], BF, tag="wq")
    wk_sb = wres.tile([128, KD, LHE], BF, tag="wk")
    wv_sb = wres.tile([128, KD, LHE], BF, tag="wv")
    wo_sb = wres.tile([128, KHE, D], BF, tag="wo")
    for src, dst in ((t["wq"], wq_sb), (t["wk"], wk_sb), (t["wv"], wv_sb)):
        nc.sync.dma_start(out=dst, in_=src.rearrange("(k p) e -> p k e", p=128))
    nc.sync.dma_start(out=wo_sb, in_=t["wo"].rearrange("(k p) e -> p k e", p=128))

    # persistent attention state: K and V for the full sequence
    attp = tc.alloc_tile_pool(name="attp", bufs=1)
    kT = attp.tile([128, LH // 2, T], BF, tag="kT")
    vS = attp.tile([128, NST, LH * 65], BF, tag="vS")
    nc.vector.memset(vS, 1.0)

    with tc.tile_pool(name="work", bufs=1) as wp, \
         tc.tile_pool(name="wstream", bufs=1) as wstr, \
         tc.tile_pool(name="scpsum", bufs=2, space="PSUM") as psc, \
         tc.tile_pool(name="oapsum", bufs=2, space="PSUM") as poa, \
         tc.tile_pool(name="genpsum", bufs=2, space="PSUM") as pgen:

        xb_tiles = {}
        hT_tiles = {}
        h2_tiles = {}
        qT_tiles = {}

        def gen_ps(tag="gen", shape=(128, 512)):
            return pgen.tile(list(shape), F32, tag="gen", bufs=2, name=tag)

        # ---------- LN helpers ----------
        def ln_stats(src_k):
            """src_k: list of KD [128,512] bf16 APs. Returns (Ab, Bb) bf16
            broadcast tiles: normalized = src*Ab + Bb."""
            ps_s = gen_ps("ps_s", (1, 512))
            ps_q = gen_ps("ps_q", (1, 512))
            for k in range(KD):
                sq = wp.tile([128, 512], BF, tag="ln_sq", bufs=2, name="ln_sq")
                nc.vector.tensor_mul(out=sq, in0=src_k[k], in1=src_k[k])
                mm(out=ps_s, lhsT=ones_col, rhs=src_k[k],
                   start=(k == 0), stop=(k == KD - 1))
                mm(out=ps_q, lhsT=ones_col, rhs=sq,
                   start=(k == 0), stop=(k == KD - 1))
            m_sb = wp.tile([1, 512], F32, tag="ln_m", bufs=1, name="ln_m")
            e2 = wp.tile([1, 512], F32, tag="ln_e2", bufs=1, name="ln_e2")
            a_t = wp.tile([1, 512], F32, tag="ln_a", bufs=1, name="ln_a")
            nc.vector.tensor_scalar_mul(out=m_sb, in0=ps_s, scalar1=1.0 / D)
            nc.vector.tensor_scalar_mul(out=e2, in0=ps_q, scalar1=1.0 / D)
            nc.vector.tensor_mul(out=a_t, in0=m_sb, in1=m_sb)      # m^2
            nc.vector.scalar_tensor_tensor(                         # var+eps
                out=e2, in0=e2, scalar=EPS, in1=a_t,
                op0=Alu.add, op1=Alu.subtract)
            nc.scalar.activation(out=e2, in_=e2, func=Act.Sqrt)     # sd
            nc.vector.reciprocal(out=a_t, in_=e2)                   # 1/sd
            nc.vector.scalar_tensor_tensor(                         # -m/sd
                out=m_sb, in0=m_sb, scalar=-1.0, in1=a_t,
                op0=Alu.mult, op1=Alu.mult)
            ac = wp.tile([1, 512], BF, tag="ln_ac", bufs=1, name="ln_ac")
            bc_ = wp.tile([1, 512], BF, tag="ln_bc", bufs=1, name="ln_bc")
            nc.vector.tensor_copy(out=ac, in_=a_t)
            nc.vector.tensor_copy(out=bc_, in_=m_sb)
            Ab = wp.tile([128, 512], BF, tag="ln_Ab", bufs=1, name="ln_Ab")
            Bb = wp.tile([128, 512], BF, tag="ln_Bb", bufs=1, name="ln_Bb")
            nc.gpsimd.partition_broadcast(Ab, ac)
            nc.gpsimd.partition_broadcast(Bb, bc_)
            return Ab, Bb

        def ln_apply(src, Ab, Bb, g_sb, be_sb, k, out_slice):
            t1 = wp.tile([128, 512], BF, tag="ln_t1", bufs=1, name="ln_t1")
            nc.vector.tensor_mul(out=t1, in0=src, in1=Ab)
            nc.vector.tensor_add(out=t1, in0=t1, in1=Bb)
            nc.vector.tensor_scalar(
                out=out_slice, in0=t1,
                scalar1=g_sb[:, k:k + 1], scalar2=be_sb[:, k:k + 1],
                op0=Alu.mult, op1=Alu.add)

        # ---------- stage functions ----------
        def ln1_load(ci):
            c0 = ci * 512
            xb = wp.tile([128, KD, 512], BF, tag="xb", bufs=1, name="xb")
            xb_tiles[ci] = xb
            nc.sync.dma_start(out=xb, in_=xb_v[:, :, c0:c0 + 512])

        def ln1_compute(ci):
            xb = xb_tiles[ci]
            src = [xb[:, k, :] for k in range(KD)]
            Ab, Bb = ln_stats(src)
            hT = wp.tile([128, KD, 512], BF, tag="hT", bufs=1, name="hT")
            hT_tiles[ci] = hT
            for k in range(KD):
                ln_apply(src[k], Ab, Bb, g1_sb, be1_sb, k, hT[:, k, :])

        def qkv(ci):
            c0 = ci * 512
            hT = hT_tiles.pop(ci)
            qT = wp.tile([128, LH // 2, 512], BF, tag="qT", bufs=1, name="qT")
            qT_tiles[ci] = qT
            for w_sb, dslice in (
                    (wk_sb, lambda et: kT[:, et, c0:c0 + 512]),
                    (wq_sb, lambda et: qT[:, et, :])):
                for et in range(LH // 2):
                    ps = gen_ps("ps_proj")
                    for k in range(KD):
                        mm(out=ps, lhsT=w_sb[:, k, et * 128:(et + 1) * 128],
                           rhs=hT[:, k, :],
                           start=(k == 0), stop=(k == KD - 1))
                    nc.vector.tensor_copy(out=dslice(et), in_=ps)
            for sti in range(4):
                st = ci * 4 + sti
                ps = gen_ps("ps_v")
                for k in range(KD):
                    mm(out=ps, lhsT=hT[:, k, sti * 128:sti * 128 + 128],
                       rhs=wv_sb[:, k, :],
                       start=(k == 0), stop=(k == KD - 1))
                nc.vector.tensor_copy(
                    out=vS[:, st, :].rearrange("p (h e) -> p h e",
                                               h=LH)[:, :, 0:64],
                    in_=ps.rearrange("p (h e) -> p h e", e=64))

        def res1_ln2(cj):
            c0 = cj * 512
            ar1v = ar1_out[cj].rearrange("(k p) t -> p k t", p=128)
            xmid = wp.tile([128, KD, 512], BF, tag="xmid", bufs=1, name="xmid")
            for k in range(KD):
                ar_sb = wp.tile([128, 512], BF, tag="arsb", bufs=2,
                                name="ar1sb")
                nc.sync.dma_start(out=ar_sb, in_=ar1v[:, k, :])
                xb2 = wp.tile([128, 512], BF, tag="xb2", bufs=2, name="xb2")
                nc.sync.dma_start(out=xb2, in_=xb_v[:, k, c0:c0 + 512])
                nc.vector.scalar_tensor_tensor(
                    out=xmid[:, k, :], in0=ar_sb,
                    scalar=bo_sb[:, k:k + 1], in1=xb2,
                    op0=Alu.add, op1=Alu.add)
                nc.sync.dma_start(out=xmid_v[:, k, c0:c0 + 512],
                                  in_=xmid[:, k, :])
            src = [xmid[:, k, :] for k in range(KD)]
            Ab2, Bb2 = ln_stats(src)
            h2 = wp.tile([128, KD, 512], BF, tag="h2", bufs=1, name="h2")
            h2_tiles[cj] = h2
            for k in range(KD):
                ln_apply(src[k], Ab2, Bb2, g2_sb, be2_sb, k, h2[:, k, :])

        def fc1_thunks(cj):
            """FC1(cj) as a list of ~4-matmul thunks (injected between
            attention pairs).  Call after res1_ln2(cj)."""
            h2 = h2_tiles.pop(cj)
            u = wp.tile([128, KFF, 512], BF, tag="u", bufs=1, name="u")
            u_box[cj] = u
            thunks = []
            w1p_tiles = {}
            ps_box = {}

            def load_w1p(p):
                def f():
                    w1p = wstr.tile([128, KD, 256], BF, tag="w1p", bufs=2,
                                    name="w1p")
                    w1p_tiles[p] = w1p
                    nc.sync.dma_start(out=w1p,
                                      in_=w1_v[:, :, p * 256:(p + 1) * 256])
                return f

            def up_half(fft, half):
                def f():
                    if half == 0:
                        ps_box[fft] = gen_ps("ps_u")
                    ps = ps_box[fft]
                    w1p = w1p_tiles[fft // 2]
                    col = (fft % 2) * 128
                    for kk in range(4):
                        k = half * 4 + kk
                        mm(out=ps, lhsT=w1p[:, k, col:col + 128],
                           rhs=h2[:, k, :],
                           start=(k == 0), stop=(k == KD - 1))
                    if half == 1:
                        nc.scalar.activation(out=u[:, fft, :], in_=ps,
                                             func=Act.Relu,
                                             bias=b1_sb[:, fft:fft + 1])
                        del ps_box[fft]
                return f

            load_w1p(0)()
            load_w1p(1)()
            for fft in range(KFF):
                if fft % 2 == 0 and 2 + fft // 2 < KFF // 2:
                    thunks.append(load_w1p(2 + fft // 2))
                thunks.append(up_half(fft, 0))
                thunks.append(up_half(fft, 1))
            return thunks

        def fc2_ar2(cj):
            """FC2(cj) as a dense matmul block + bf16 pair AllReduce."""
            u = u_box.pop(cj)
            w2p_tiles = {}

            def load_w2p(p):
                w2p = wstr.tile([128, KFF, 128], BF, tag="w2p", bufs=2,
                                name="w2p")
                w2p_tiles[p] = w2p
                nc.sync.dma_start(out=w2p,
                                  in_=w2_v[:, :, p * 128:(p + 1) * 128])

            load_w2p(0)
            load_w2p(1)
            for dt in range(KD):
                if 2 + dt < KD:
                    load_w2p(2 + dt)
                ps = gen_ps("ps_f")
                for k2 in range(KFF):
                    mm(out=ps, lhsT=w2p_tiles[dt][:, k2, :],
                       rhs=u[:, k2, :],
                       start=(k2 == 0), stop=(k2 == KFF - 1))
                stg = wp.tile([128, 512], BF, tag="stg", bufs=2, name="stg2")
                nc.vector.tensor_copy(out=stg, in_=ps)
                nc.sync.dma_start(
                    out=ar2_in[cj][dt * 128:(dt + 1) * 128, :], in_=stg)
            nc.gpsimd.collective_compute(
                "AllReduce", Alu.add, replica_groups=PAIRS,
                ins=[ar2_in[cj].opt()], outs=[ar2_out[cj].opt()])

        def wo_ar1(ci, oT):
            for dt in range(KD):
                ps = gen_ps("ps_wo")
                for k in range(KHE):
                    mm(out=ps, lhsT=wo_sb[:, k, dt * 128:(dt + 1) * 128],
                       rhs=oT[:, k, :],
                       start=(k == 0), stop=(k == KHE - 1))
                stg = wp.tile([128, 512], BF, tag="stg", bufs=2, name="stg1")
                nc.vector.tensor_copy(out=stg, in_=ps)
                nc.sync.dma_start(
                    out=ar1_in[ci][dt * 128:(dt + 1) * 128, :], in_=stg)
            nc.gpsimd.collective_compute(
                "AllReduce", Alu.add, replica_groups=PAIRS,
                ins=[ar1_in[ci].opt()], outs=[ar1_out[ci].opt()])

        def res2(cj):
            c0 = cj * 512
            ar2v = ar2_out[cj].rearrange("(k p) t -> p k t", p=128)
            for k in range(KD):
                a2 = wp.tile([128, 512], BF, tag="arsb", bufs=2, name="ar2sb")
                nc.sync.dma_start(out=a2, in_=ar2v[:, k, :])
                xm = wp.tile([128, 512], BF, tag="xm2", bufs=2, name="xm2")
                nc.sync.dma_start(out=xm, in_=xmid_v[:, k, c0:c0 + 512])
                o_f = wp.tile([128, 512], F32, tag="o_f", bufs=2, name="o_f")
                nc.vector.scalar_tensor_tensor(
                    out=o_f, in0=a2, scalar=b2_sb[:, k:k + 1], in1=xm,
                    op0=Alu.add, op1=Alu.add)
                nc.sync.dma_start(
                    out=outT_v[k * 128:(k + 1) * 128, c0:c0 + 512], in_=o_f)

        u_box = {}

        # ---------- attention for one chunk (head-interleaved) ----------
        def att_block(ci):
            nb = 4 * (ci + 1)
            ngrp = nb // 2
            qT = qT_tiles.pop(ci)
            dn8 = wp.tile([LH, 512], F32, tag="dn8", bufs=1, name="dn8")
            oT = wp.tile([128, KHE, 512], BF, tag="oT", bufs=1, name="oT")
            ou_map = {}
            ex_map = {}
            po_map = {}
            pending = []
            state = {"i": 0, "per": 0}

            def inject():
                n = min(state["per"], len(pending) - state["i"])
                for _ in range(n):
                    pending[state["i"]]()
                    state["i"] += 1

            def emit_S(h, g):
                hp, hi = h // 2, h % 2
                e0 = hi * 64
                grp = psc.tile([128, 2, 512], F32, tag="ps_sc", bufs=2,
                               name="ps_sc")
                for j in range(2):
                    sb = 2 * g + j
                    mm(out=grp[:, j, :],
                       lhsT=kT[e0:e0 + 64, hp, sb * 128:(sb + 1) * 128],
                       rhs=qT[e0:e0 + 64, hp, :],
                       start=True, stop=True)
                exg = wp.tile([128, 2, 512], BF, tag="ex", bufs=9, name="ex")
                ex_map[(h, g)] = exg
                nc.scalar.activation(out=exg, in_=grp, func=Act.Exp,
                                     scale=float(HS) ** -0.5)
                base = 4 * ci
                for j in range(2):
                    sb = 2 * g + j
                    if sb >= base:
                        nc.vector.tensor_mul(out=exg[:, j, :],
                                             in0=exg[:, j, :],
                                             in1=masks[sb - base])

            def emit_AV(h, g):
                if g == 0:
                    po_map[h] = poa.tile([65, 512], F32, tag="po", bufs=2,
                                         name="po")
                po = po_map[h]
                exg = ex_map.pop((h, g))
                for j in range(2):
                    sb = 2 * g + j
                    mm(out=po, lhsT=vS[:, sb, h * 65:h * 65 + 65],
                       rhs=exg[:, j, :],
                       start=(sb == 0), stop=(sb == nb - 1))
                if g == ngrp - 1:
                    ou = wp.tile([64, 512], BF, tag="ou", bufs=8, name="ou")
                    ou_map[h] = ou
                    nc.vector.tensor_copy(out=ou, in_=po[0:64, :])
                    dnr = wp.tile([1, 512], F32, tag="dnr", bufs=2,
                                  name="dnr")
                    nc.vector.tensor_copy(out=dnr, in_=po[64:65, :])
                    nc.sync.dma_start(out=dn8[h:h + 1, :], in_=dnr)

            for h in range(LH):
                for g in range(ngrp):
                    emit_S(h, g)
                    if h > 0:
                        emit_AV(h - 1, g)
                    inject()
                if h == 1 and ci >= 1:
                    res1_ln2(ci - 1)
                    pending.extend(fc1_thunks(ci - 1))
                    # spread thunks over remaining pair slots (+norm window)
                    slots = (LH - 1 - h) * ngrp + ngrp + 8
                    state["per"] = -(-len(pending) // max(slots, 1))
            for g in range(ngrp):
                emit_AV(LH - 1, g)
                inject()

            # normalization: o /= rowsum(exp); remaining thunks keep PE busy
            rec8 = wp.tile([LH, 512], F32, tag="rec8", bufs=1, name="rec8")
            nc.vector.reciprocal(out=rec8, in_=dn8)
            rb8 = wp.tile([LH, 512], BF, tag="rb8", bufs=1, name="rb8")
            nc.vector.tensor_copy(out=rb8, in_=rec8)
            for h in range(LH):
                rbt = wp.tile([1, 512], BF, tag="rbt", bufs=2, name="rbt")
                nc.sync.dma_start(out=rbt, in_=rb8[h:h + 1, :])
                bc = wp.tile([64, 512], BF, tag="bc", bufs=2, name="bc")
                nc.gpsimd.partition_broadcast(bc, rbt)
                nc.vector.tensor_mul(
                    out=oT[(h % 2) * 64:(h % 2) * 64 + 64, h // 2, :],
                    in0=ou_map[h], in1=bc)
                inject()
            state["per"] = len(pending)
            inject()
            return oT

        # ---------- main schedule ----------
        ln1_load(0)
        ln1_compute(0)
        for ci in range(NCH):
            qkv(ci)
            if ci + 1 < NCH:
                ln1_load(ci + 1)
            oT = att_block(ci)          # + res1_ln2(ci-1) + FC1(ci-1)
            wo_ar1(ci, oT)
            if ci + 1 < NCH:
                ln1_compute(ci + 1)
            if ci >= 1:
                fc2_ar2(ci - 1)
            if ci >= 2:
                res2(ci - 2)
        # ---------- tail ----------
        res1_ln2(NCH - 1)
        for th in fc1_thunks(NCH - 1):
            th()
        fc2_ar2(NCH - 1)
        res2(NCH - 2)
        res2(NCH - 1)

    attp.release()
    wres.release()
    consts.release()
    dram.release()


def _build():
    nc = bacc.Bacc("TRN2", target_bir_lowering=False, debug=False,
                   num_devices=NCORES)

    tensors = {}
    tensors["xbT"] = nc.dram_tensor("xbT", [D, T], BF,
                                    kind="ExternalInput").ap()
    for name, shape, dt in (
        ("wq", [D, LHE], BF), ("wk", [D, LHE], BF), ("wv", [D, LHE], BF),
        ("wo", [LHE, D], BF), ("w1", [D, LFF], BF), ("w2", [LFF, D], BF),
        ("b1l", [LFF], F32), ("bo", [D], F32), ("b2", [D], F32),
        ("g1", [D], F32), ("be1", [D], F32), ("g2", [D], F32),
        ("be2", [D], F32),
    ):
        tensors[name] = nc.dram_tensor(name, shape, dt,
                                       kind="ExternalInput").ap()
    tensors["outT"] = nc.dram_tensor("out", [D, T], F32,
                                     kind="ExternalOutput").ap()

    with tile.TileContext(nc, num_cores=NCORES) as tc:
        _emit(nc, tc, tensors)

    nc.compile()
    return nc


_NC_CACHE = None


def _get_nc():
    global _NC_CACHE
    if _NC_CACHE is None:
        _NC_CACHE = _build()
    return _NC_CACHE


def _shard_inputs(x, Wq, Wk, Wv, Wo, bo, W1, b1, W2, b2, g1, be1, g2, be2):
    """Build the 8 per-core input maps."""
    bf = lambda a: np.ascontiguousarray(a).astype(BF16NP)
    f32 = lambda a: np.ascontiguousarray(a, dtype=np.float32)

    in_maps = []
    for c in range(NCORES):
        b, half = divmod(c, TP)
        heads = slice(half * LH, (half + 1) * LH)
        ffs = slice(half * LFF, (half + 1) * LFF)
        hes = slice(half * LHE, (half + 1) * LHE)
        wq_l = np.concatenate(list(np.asarray(Wq)[heads]), axis=1)
        wk_l = np.concatenate(list(np.asarray(Wk)[heads]), axis=1)
        wv_l = np.concatenate(list(np.asarray(Wv)[heads]), axis=1)
        in_maps.append({
            "xbT": bf(np.asarray(x)[b].T),
            "wq": bf(wq_l), "wk": bf(wk_l), "wv": bf(wv_l),
            "wo": bf(np.asarray(Wo)[hes, :]),
            "w1": bf(np.asarray(W1)[:, ffs]), "w2": bf(np.asarray(W2)[ffs, :]),
            "b1l": f32(np.asarray(b1)[ffs]),
            "bo": f32(bo), "b2": f32(b2),
            "g1": f32(g1), "be1": f32(be1), "g2": f32(g2), "be2": f32(be2),
        })
    return in_maps


def kernel(x, Wq, Wk, Wv, Wo, bo, W1, b1, W2, b2, g1, be1, g2, be2,
           _trace=False):
    nc = _get_nc()
    in_maps = _shard_inputs(x, Wq, Wk, Wv, Wo, bo, W1, b1, W2, b2,
                            g1, be1, g2, be2)
    res = run_bass_kernel_spmd(nc, in_maps, list(range(NCORES)),
                               trace=_trace)
    out = np.empty((B, T, D), dtype=np.float32)
    for b in range(B):
        out[b] = res.results[TP * b]["out"].T
    if _trace:
        kernel.last_exec_time_ns = res.exec_time_ns
        kernel.last_results = res
    return out


# revision 11
# speedup vs baseline: 1.2549x; 1.0332x over previous
"""Trainium2 Bass kernel for a pre-LN transformer block (B=4, T=2048, D=1024,
H=16, HS=64, FF=4096, causal attention).

Sharding: data-parallel over batch pairs x 2-way tensor-parallel
(heads for attention, columns/rows for FFN) with a pair AllReduce (bf16)
after the attention output projection and after FC2 (Megatron style).

Core c (0..7): batch b = c//2, TP half = c%2 (8 local heads, 2048 local FF).
Activations are feature-major on chip (d on partitions, t on free dim);
the host transposes x in (bf16) and the output back.

Schedule per 512-token chunk-slot ci (software-pipelined, depth 2-3):
  QKV(ci) -> attention(ci) -> Wo(ci)+AR1(ci) -> LN1(ci+1) -> FC2(ci-1)+AR2(ci-1)
  -> residual2(ci-2)
Attention interleaves the score matmuls of head h with the AV matmuls of
head h-1 so the PE never waits on exp() (which runs one head behind on ACT),
and FC1(ci-1) thunks are injected between score/AV pairs to absorb the
ACT-vs-PE rate gap.  residual1+LN2(ci-1) is emitted two heads into
attention(ci) so its DVE work never blocks and AR1 latency is hidden.
"""

import numpy as np
import ml_dtypes

import concourse.bacc as bacc
import concourse.bass as bass
import concourse.mybir as mybir
import concourse.tile as tile
from concourse.bass_utils import run_bass_kernel_spmd

BF16NP = ml_dtypes.bfloat16

B, T, D, H, HS, FF = 4, 2048, 1024, 16, 64, 4096
EPS = 1e-5
NCORES = 8
TP = 2
LH = H // TP          # 8 local heads
LHE = LH * HS         # 512 local head-embed width
LFF = FF // TP        # 2048 local FF
KD = D // 128         # 8 d k-tiles
KHE = LHE // 128      # 4 he k-tiles
KFF = LFF // 128      # 16 ff k-tiles
NCH = T // 512        # 4 t-chunks of 512
NST = T // 128        # 16 s-tiles of 128
PAIRS = [[0, 1], [2, 3], [4, 5], [6, 7]]

F32 = mybir.dt.float32
BF = mybir.dt.bfloat16


def _emit(nc, tc, t):
    mm = nc.tensor.matmul
    Alu = mybir.AluOpType
    Act = mybir.ActivationFunctionType

    xb_v = t["xbT"].rearrange("(k p) t -> p k t", p=128)
    w1_v = t["w1"].rearrange("(k p) e -> p k e", p=128)
    w2_v = t["w2"].rearrange("(k p) e -> p k e", p=128)
    outT_v = t["outT"]

    # ---------------- persistent pools ----------------
    dram = tc.alloc_tile_pool(name="dram", bufs=1, space="DRAM")
    ar1_in = [dram.tile([D, 512], BF, name=f"ar1i{c}") for c in range(NCH)]
    ar1_out = [dram.tile([D, 512], BF, name=f"ar1o{c}") for c in range(NCH)]
    ar2_in = [dram.tile([D, 512], BF, name=f"ar2i{c}") for c in range(NCH)]
    ar2_out = [dram.tile([D, 512], BF, name=f"ar2o{c}") for c in range(NCH)]
    xmid_dram = dram.tile([D, T], BF, name="xmidd")
    xmid_v = xmid_dram.rearrange("(k p) t -> p k t", p=128)

    consts = tc.alloc_tile_pool(name="consts", bufs=1)
    ones_col = consts.tile([128, 1], BF)
    nc.vector.memset(ones_col, 1.0)

    g1_sb = consts.tile([128, KD], F32)
    be1_sb = consts.tile([128, KD], F32)
    g2_sb = consts.tile([128, KD], F32)
    be2_sb = consts.tile([128, KD], F32)
    bo_sb = consts.tile([128, KD], F32)
    b2_sb = consts.tile([128, KD], F32)
    b1_sb = consts.tile([128, KFF], F32)
    for name, dst in (("g1", g1_sb), ("be1", be1_sb), ("g2", g2_sb),
                      ("be2", be2_sb), ("bo", bo_sb), ("b2", b2_sb)):
        nc.sync.dma_start(out=dst, in_=t[name].rearrange("(k p) -> p k", p=128))
    nc.sync.dma_start(out=b1_sb, in_=t["b1l"].rearrange("(k p) -> p k", p=128))

    masks = []
    for midx in range(4):
        mk = consts.tile([128, 512], BF, name=f"mask{midx}")
        nc.vector.memset(mk, 1.0)
        nc.gpsimd.affine_select(
            out=mk, in_=mk, compare_op=Alu.is_ge, fill=0.0,
            base=-(midx * 128), channel_multiplier=-1, pattern=[[1, 512]])
        masks.append(mk)

    # resident weights: QKV + Wo
    wres = tc.alloc_tile_pool(name="wres", bufs=1)
    wq_sb = wres.tile([128, KD, LHE], BF, tag="wq")
    wk_sb = wres.tile([128, KD, LHE], BF, tag="wk")
    wv_sb = wres.tile([128, KD, LHE], BF, tag="wv")
    wo_sb = wres.tile([128, KHE, D], BF, tag="wo")
    for src, dst in ((t["wq"], wq_sb), (t["wk"], wk_sb), (t["wv"], wv_sb)):
        nc.sync.dma_start(out=dst, in_=src.rearrange("(k p) e -> p k e", p=128))
    nc.sync.dma_start(out=wo_sb, in_=t["wo"].rearrange("(k p) e -> p k e", p=128))

    # persistent attention state: K and V for the full sequence
    attp = tc.alloc_tile_pool(name="attp", bufs=1)
    kT = attp.tile([128, LH // 2, T], BF, tag="kT")
    vS = attp.tile([128, NST, LH * 65], BF, tag="vS")
    nc.vector.memset(vS, 1.0)

    with tc.tile_pool(name="work", bufs=1) as wp, \
         tc.tile_pool(name="wstream", bufs=1) as wstr, \
         tc.tile_pool(name="scpsum", bufs=2, space="PSUM") as psc, \
         tc.tile_pool(name="oapsum", bufs=2, space="PSUM") as poa, \
         tc.tile_pool(name="genpsum", bufs=2, space="PSUM") as pgen:

        xb_tiles = {}
        hT_tiles = {}
        h2_tiles = {}
        qT_tiles = {}

        def gen_ps(tag="gen", shape=(128, 512)):
            return pgen.tile(list(shape), F32, tag="gen", bufs=2, name=tag)

        # ---------- LN helpers ----------
        def ln_stats(src_k):
            """src_k: list of KD [128,512] bf16 APs. Returns (Ab, Bb) bf16
            broadcast tiles: normalized = src*Ab + Bb."""
            ps_s = gen_ps("ps_s", (1, 512))
            ps_q = gen_ps("ps_q", (1, 512))
            for k in range(KD):
                sq = wp.tile([128, 512], BF, tag="ln_sq", bufs=2, name="ln_sq")
                nc.vector.tensor_mul(out=sq, in0=src_k[k], in1=src_k[k])
                mm(out=ps_s, lhsT=ones_col, rhs=src_k[k],
                   start=(k == 0), stop=(k == KD - 1))
                mm(out=ps_q, lhsT=ones_col, rhs=sq,
                   start=(k == 0), stop=(k == KD - 1))
            return ln_chain(ps_s, ps_q)

        def ln_chain(ps_s, ps_q):
            m_sb = wp.tile([1, 512], F32, tag="ln_m", bufs=1, name="ln_m")
            e2 = wp.tile([1, 512], F32, tag="ln_e2", bufs=1, name="ln_e2")
            a_t = wp.tile([1, 512], F32, tag="ln_a", bufs=1, name="ln_a")
            nc.vector.tensor_scalar_mul(out=m_sb, in0=ps_s, scalar1=1.0 / D)
            nc.vector.tensor_scalar_mul(out=e2, in0=ps_q, scalar1=1.0 / D)
            nc.vector.tensor_mul(out=a_t, in0=m_sb, in1=m_sb)      # m^2
            nc.vector.scalar_tensor_tensor(                         # var+eps
                out=e2, in0=e2, scalar=EPS, in1=a_t,
                op0=Alu.add, op1=Alu.subtract)
            nc.scalar.activation(out=e2, in_=e2, func=Act.Sqrt)     # sd
            nc.vector.reciprocal(out=a_t, in_=e2)                   # 1/sd
            nc.vector.scalar_tensor_tensor(                         # -m/sd
                out=m_sb, in0=m_sb, scalar=-1.0, in1=a_t,
                op0=Alu.mult, op1=Alu.mult)
            ac = wp.tile([1, 512], BF, tag="ln_ac", bufs=1, name="ln_ac")
            bc_ = wp.tile([1, 512], BF, tag="ln_bc", bufs=1, name="ln_bc")
            nc.vector.tensor_copy(out=ac, in_=a_t)
            nc.vector.tensor_copy(out=bc_, in_=m_sb)
            Ab = wp.tile([128, 512], BF, tag="ln_Ab", bufs=1, name="ln_Ab")
            Bb = wp.tile([128, 512], BF, tag="ln_Bb", bufs=1, name="ln_Bb")
            nc.gpsimd.partition_broadcast(Ab, ac)
            nc.gpsimd.partition_broadcast(Bb, bc_)
            return Ab, Bb

        def ln_apply(src, Ab, Bb, g_sb, be_sb, k, out_slice):
            t1 = wp.tile([128, 512], BF, tag="ln_t1", bufs=1, name="ln_t1")
            nc.vector.tensor_mul(out=t1, in0=src, in1=Ab)
            nc.vector.tensor_add(out=t1, in0=t1, in1=Bb)
            nc.vector.tensor_scalar(
                out=out_slice, in0=t1,
                scalar1=g_sb[:, k:k + 1], scalar2=be_sb[:, k:k + 1],
                op0=Alu.mult, op1=Alu.add)

        # ---------- stage functions ----------
        def ln1_load(ci):
            c0 = ci * 512
            xb = wp.tile([128, KD, 512], BF, tag="xb", bufs=1, name="xb")
            xb_tiles[ci] = xb
            nc.sync.dma_start(out=xb, in_=xb_v[:, :, c0:c0 + 512])

        def ln1_compute(ci):
            xb = xb_tiles[ci]
            src = [xb[:, k, :] for k in range(KD)]
            Ab, Bb = ln_stats(src)
            hT = wp.tile([128, KD, 512], BF, tag="hT", bufs=1, name="hT")
            hT_tiles[ci] = hT
            for k in range(KD):
                ln_apply(src[k], Ab, Bb, g1_sb, be1_sb, k, hT[:, k, :])

        def qkv(ci):
            c0 = ci * 512
            hT = hT_tiles.pop(ci)
            qT = wp.tile([128, LH // 2, 512], BF, tag="qT", bufs=1, name="qT")
            qT_tiles[ci] = qT
            for w_sb, dslice in (
                    (wk_sb, lambda et: kT[:, et, c0:c0 + 512]),
                    (wq_sb, lambda et: qT[:, et, :])):
                for et in range(LH // 2):
                    ps = gen_ps("ps_proj")
                    for k in range(KD):
                        mm(out=ps, lhsT=w_sb[:, k, et * 128:(et + 1) * 128],
                           rhs=hT[:, k, :],
                           start=(k == 0), stop=(k == KD - 1))
                    nc.vector.tensor_copy(out=dslice(et), in_=ps)
            for sti in range(4):
                st = ci * 4 + sti
                ps = gen_ps("ps_v")
                for k in range(KD):
                    mm(out=ps, lhsT=hT[:, k, sti * 128:sti * 128 + 128],
                       rhs=wv_sb[:, k, :],
                       start=(k == 0), stop=(k == KD - 1))
                nc.vector.tensor_copy(
                    out=vS[:, st, :].rearrange("p (h e) -> p h e",
                                               h=LH)[:, :, 0:64],
                    in_=ps.rearrange("p (h e) -> p h e", e=64))

        def res1_ln2(cj):
            """residual1 + LN2, fused per k-tile: the stats matmuls pace one
            tile behind the DVE residual chain instead of after all of it."""
            c0 = cj * 512
            ar1v = ar1_out[cj].rearrange("(k p) t -> p k t", p=128)
            xmid = wp.tile([128, KD, 512], BF, tag="xmid", bufs=1, name="xmid")
            ps_s = gen_ps("ps_s", (1, 512))
            ps_q = gen_ps("ps_q", (1, 512))
            for k in range(KD):
                ar_sb = wp.tile([128, 512], BF, tag="arsb", bufs=2,
                                name="ar1sb")
                nc.sync.dma_start(out=ar_sb, in_=ar1v[:, k, :])
                xb2 = wp.tile([128, 512], BF, tag="xb2", bufs=2, name="xb2")
                nc.sync.dma_start(out=xb2, in_=xb_v[:, k, c0:c0 + 512])
                nc.vector.scalar_tensor_tensor(
                    out=xmid[:, k, :], in0=ar_sb,
                    scalar=bo_sb[:, k:k + 1], in1=xb2,
                    op0=Alu.add, op1=Alu.add)
                nc.sync.dma_start(out=xmid_v[:, k, c0:c0 + 512],
                                  in_=xmid[:, k, :])
                sq = wp.tile([128, 512], BF, tag="ln_sq", bufs=2, name="ln_sq")
                nc.vector.tensor_mul(out=sq, in0=xmid[:, k, :],
                                     in1=xmid[:, k, :])
                mm(out=ps_s, lhsT=ones_col, rhs=xmid[:, k, :],
                   start=(k == 0), stop=(k == KD - 1))
                mm(out=ps_q, lhsT=ones_col, rhs=sq,
                   start=(k == 0), stop=(k == KD - 1))
            Ab2, Bb2 = ln_chain(ps_s, ps_q)
            h2 = wp.tile([128, KD, 512], BF, tag="h2", bufs=1, name="h2")
            h2_tiles[cj] = h2
            for k in range(KD):
                ln_apply(xmid[:, k, :], Ab2, Bb2, g2_sb, be2_sb, k,
                         h2[:, k, :])

        def fc1_thunks(cj):
            """FC1(cj) as a list of ~4-matmul thunks (injected between
            attention pairs).  Call after res1_ln2(cj)."""
            h2 = h2_tiles.pop(cj)
            u = wp.tile([128, KFF, 512], BF, tag="u", bufs=1, name="u")
            u_box[cj] = u
            thunks = []
            w1p_tiles = {}
            ps_box = {}

            def load_w1p(p):
                def f():
                    w1p = wstr.tile([128, KD, 256], BF, tag="w1p", bufs=2,
                                    name="w1p")
                    w1p_tiles[p] = w1p
                    nc.sync.dma_start(out=w1p,
                                      in_=w1_v[:, :, p * 256:(p + 1) * 256])
                return f

            def up_half(fft, half):
                def f():
                    if half == 0:
                        ps_box[fft] = gen_ps("ps_u")
                    ps = ps_box[fft]
                    w1p = w1p_tiles[fft // 2]
                    col = (fft % 2) * 128
                    for kk in range(4):
                        k = half * 4 + kk
                        mm(out=ps, lhsT=w1p[:, k, col:col + 128],
                           rhs=h2[:, k, :],
                           start=(k == 0), stop=(k == KD - 1))
                    if half == 1:
                        nc.scalar.activation(out=u[:, fft, :], in_=ps,
                                             func=Act.Relu,
                                             bias=b1_sb[:, fft:fft + 1])
                        del ps_box[fft]
                return f

            load_w1p(0)()
            load_w1p(1)()
            for fft in range(KFF):
                if fft % 2 == 0 and 2 + fft // 2 < KFF // 2:
                    thunks.append(load_w1p(2 + fft // 2))
                thunks.append(up_half(fft, 0))
                thunks.append(up_half(fft, 1))
            return thunks

        def fc2_prefetch(cj):
            """Issue the first two W2 piece DMAs early (right after Wo)."""
            box = {}

            def load_w2p(p):
                w2p = wstr.tile([128, KFF, 128], BF, tag="w2p", bufs=2,
                                name="w2p")
                box[p] = w2p
                nc.sync.dma_start(out=w2p,
                                  in_=w2_v[:, :, p * 128:(p + 1) * 128])

            box["load"] = load_w2p
            load_w2p(0)
            load_w2p(1)
            w2p_box[cj] = box

        def fc2_ar2(cj, split=False):
            """FC2(cj) as a dense matmul block + bf16 pair AllReduce.
            split=True: two half-AllReduces so the tail can overlap."""
            u = u_box.pop(cj)
            box = w2p_box.pop(cj)
            load_w2p = box["load"]
            halves = ((0, 4), (4, 8)) if split else ((0, 8),)
            for hi, (d0, d1) in enumerate(halves):
                for dt in range(d0, d1):
                    if 2 + dt < KD:
                        load_w2p(2 + dt)
                    ps = gen_ps("ps_f")
                    for k2 in range(KFF):
                        mm(out=ps, lhsT=box[dt][:, k2, :],
                           rhs=u[:, k2, :],
                           start=(k2 == 0), stop=(k2 == KFF - 1))
                    stg = wp.tile([128, 512], BF, tag="stg", bufs=2,
                                  name="stg2")
                    nc.vector.tensor_copy(out=stg, in_=ps)
                    nc.sync.dma_start(
                        out=ar2_in[cj][dt * 128:(dt + 1) * 128, :], in_=stg)
                if split:
                    r0, r1 = d0 * 128, d1 * 128
                    nc.gpsimd.collective_compute(
                        "AllReduce", Alu.add, replica_groups=PAIRS,
                        ins=[ar2_in[cj][r0:r1, :].opt()],
                        outs=[ar2_out[cj][r0:r1, :].opt()])
            if not split:
                nc.gpsimd.collective_compute(
                    "AllReduce", Alu.add, replica_groups=PAIRS,
                    ins=[ar2_in[cj].opt()], outs=[ar2_out[cj].opt()])

        def wo_ar1(ci, oT, split=False):
            halves = ((0, 4), (4, 8)) if split else ((0, 8),)
            for hi, (d0, d1) in enumerate(halves):
                for dt in range(d0, d1):
                    ps = gen_ps("ps_wo")
                    for k in range(KHE):
                        mm(out=ps, lhsT=wo_sb[:, k, dt * 128:(dt + 1) * 128],
                           rhs=oT[:, k, :],
                           start=(k == 0), stop=(k == KHE - 1))
                    stg = wp.tile([128, 512], BF, tag="stg", bufs=2,
                                  name="stg1")
                    nc.vector.tensor_copy(out=stg, in_=ps)
                    nc.sync.dma_start(
                        out=ar1_in[ci][dt * 128:(dt + 1) * 128, :], in_=stg)
                if split:
                    r0, r1 = d0 * 128, d1 * 128
                    nc.gpsimd.collective_compute(
                        "AllReduce", Alu.add, replica_groups=PAIRS,
                        ins=[ar1_in[ci][r0:r1, :].opt()],
                        outs=[ar1_out[ci][r0:r1, :].opt()])
            if not split:
                nc.gpsimd.collective_compute(
                    "AllReduce", Alu.add, replica_groups=PAIRS,
                    ins=[ar1_in[ci].opt()], outs=[ar1_out[ci].opt()])

        def res2(cj, ks=range(KD)):
            c0 = cj * 512
            ar2v = ar2_out[cj].rearrange("(k p) t -> p k t", p=128)
            for k in ks:
                a2 = wp.tile([128, 512], BF, tag="arsb", bufs=2, name="ar2sb")
                nc.sync.dma_start(out=a2, in_=ar2v[:, k, :])
                xm = wp.tile([128, 512], BF, tag="xm2", bufs=2, name="xm2")
                nc.sync.dma_start(out=xm, in_=xmid_v[:, k, c0:c0 + 512])
                o_f = wp.tile([128, 512], F32, tag="o_f", bufs=2, name="o_f")
                nc.vector.scalar_tensor_tensor(
                    out=o_f, in0=a2, scalar=b2_sb[:, k:k + 1], in1=xm,
                    op0=Alu.add, op1=Alu.add)
                nc.sync.dma_start(
                    out=outT_v[k * 128:(k + 1) * 128, c0:c0 + 512], in_=o_f)

        u_box = {}
        w2p_box = {}

        # ---------- attention for one chunk (head-interleaved) ----------
        def att_block(ci, res1_head=1, extra_hooks=None):
            nb = 4 * (ci + 1)
            ngrp = nb // 2
            qT = qT_tiles.pop(ci)
            dn8 = wp.tile([LH, 512], F32, tag="dn8", bufs=1, name="dn8")
            oT = wp.tile([128, KHE, 512], BF, tag="oT", bufs=1, name="oT")
            ou_map = {}
            ex_map = {}
            po_map = {}
            pending = []
            state = {"i": 0, "per": 0, "reserve": 0}

            def inject(ignore_reserve=False):
                lim = len(pending) - (0 if ignore_reserve else
                                      state["reserve"])
                n = min(state["per"], lim - state["i"])
                for _ in range(max(n, 0)):
                    pending[state["i"]]()
                    state["i"] += 1

            def emit_S(h, g):
                hp, hi = h // 2, h % 2
                e0 = hi * 64
                grp = psc.tile([128, 2, 512], F32, tag="ps_sc", bufs=2,
                               name="ps_sc")
                for j in range(2):
                    sb = 2 * g + j
                    mm(out=grp[:, j, :],
                       lhsT=kT[e0:e0 + 64, hp, sb * 128:(sb + 1) * 128],
                       rhs=qT[e0:e0 + 64, hp, :],
                       start=True, stop=True)
                exg = wp.tile([128, 2, 512], BF, tag="ex", bufs=9, name="ex")
                ex_map[(h, g)] = exg
                nc.scalar.activation(out=exg, in_=grp, func=Act.Exp,
                                     scale=float(HS) ** -0.5)
                base = 4 * ci
                for j in range(2):
                    sb = 2 * g + j
                    if sb >= base:
                        nc.vector.tensor_mul(out=exg[:, j, :],
                                             in0=exg[:, j, :],
                                             in1=masks[sb - base])

            def emit_AV(h, g):
                if g == 0:
                    po_map[h] = poa.tile([65, 512], F32, tag="po", bufs=2,
                                         name="po")
                po = po_map[h]
                exg = ex_map.pop((h, g))
                for j in range(2):
                    sb = 2 * g + j
                    mm(out=po, lhsT=vS[:, sb, h * 65:h * 65 + 65],
                       rhs=exg[:, j, :],
                       start=(sb == 0), stop=(sb == nb - 1))
                if g == ngrp - 1:
                    ou = wp.tile([64, 512], BF, tag="ou", bufs=8, name="ou")
                    ou_map[h] = ou
                    nc.vector.tensor_copy(out=ou, in_=po[0:64, :])
                    dnr = wp.tile([1, 512], F32, tag="dnr", bufs=2,
                                  name="dnr")
                    nc.vector.tensor_copy(out=dnr, in_=po[64:65, :])
                    nc.sync.dma_start(out=dn8[h:h + 1, :], in_=dnr)

            for h in range(LH):
                for g in range(ngrp):
                    emit_S(h, g)
                    if h > 0:
                        emit_AV(h - 1, g)
                    inject()
                if h == res1_head and ci >= 1:
                    res1_ln2(ci - 1)
                    pending.extend(fc1_thunks(ci - 1))
                    # spread thunks over remaining pair slots, reserving a
                    # few for the normalization window
                    state["reserve"] = 10
                    slots = (LH - 1 - h) * ngrp + ngrp
                    state["per"] = max(1, -(-(len(pending) - 10)
                                            // max(slots, 1)))
                if extra_hooks and h in extra_hooks:
                    extra_hooks[h]()
            for g in range(ngrp):
                emit_AV(LH - 1, g)
                inject()

            # normalization: o /= rowsum(exp); remaining thunks keep PE busy
            rec8 = wp.tile([LH, 512], F32, tag="rec8", bufs=1, name="rec8")
            nc.vector.reciprocal(out=rec8, in_=dn8)
            rb8 = wp.tile([LH, 512], BF, tag="rb8", bufs=1, name="rb8")
            nc.vector.tensor_copy(out=rb8, in_=rec8)
            for h in range(LH):
                rbt = wp.tile([1, 512], BF, tag="rbt", bufs=2, name="rbt")
                nc.sync.dma_start(out=rbt, in_=rb8[h:h + 1, :])
                bc = wp.tile([64, 512], BF, tag="bc", bufs=2, name="bc")
                nc.gpsimd.partition_broadcast(bc, rbt)
                nc.vector.tensor_mul(
                    out=oT[(h % 2) * 64:(h % 2) * 64 + 64, h // 2, :],
                    in0=ou_map[h], in1=bc)
                state["per"] = 2
                inject(ignore_reserve=True)
            state["per"] = len(pending)
            inject(ignore_reserve=True)
            return oT

        # ---------- main schedule ----------
        ln1_load(0)
        ln1_compute(0)
        for ci in range(NCH):
            qkv(ci)
            if ci + 1 < NCH:
                ln1_load(ci + 1)
            extra = {5: (lambda: ln1_compute(1))} if ci == 0 else None
            oT = att_block(ci, res1_head=(2 if ci == 1 else 1),
                           extra_hooks=extra)
            last = ci == NCH - 1
            wo_ar1(ci, oT, split=last)
            if ci >= 1:
                fc2_prefetch(ci - 1)
            if 1 <= ci < NCH - 1:
                ln1_compute(ci + 1)
            if ci >= 1:
                fc2_ar2(ci - 1)
            if ci >= 2:
                res2(ci - 2)
        # ---------- tail ----------
        res1_ln2(NCH - 1)
        fc2_prefetch(NCH - 1)
        for th in fc1_thunks(NCH - 1):
            th()
        res2(NCH - 2)
        fc2_ar2(NCH - 1, split=True)
        res2(NCH - 1, ks=range(0, 4))
        res2(NCH - 1, ks=range(4, KD))

    attp.release()
    wres.release()
    consts.release()
    dram.release()


def _build():
    nc = bacc.Bacc("TRN2", target_bir_lowering=False, debug=False,
                   num_devices=NCORES)

    tensors = {}
    tensors["xbT"] = nc.dram_tensor("xbT", [D, T], BF,
                                    kind="ExternalInput").ap()
    for name, shape, dt in (
        ("wq", [D, LHE], BF), ("wk", [D, LHE], BF), ("wv", [D, LHE], BF),
        ("wo", [LHE, D], BF), ("w1", [D, LFF], BF), ("w2", [LFF, D], BF),
        ("b1l", [LFF], F32), ("bo", [D], F32), ("b2", [D], F32),
        ("g1", [D], F32), ("be1", [D], F32), ("g2", [D], F32),
        ("be2", [D], F32),
    ):
        tensors[name] = nc.dram_tensor(name, shape, dt,
                                       kind="ExternalInput").ap()
    tensors["outT"] = nc.dram_tensor("out", [D, T], F32,
                                     kind="ExternalOutput").ap()

    with tile.TileContext(nc, num_cores=NCORES) as tc:
        _emit(nc, tc, tensors)

    nc.compile()
    return nc


_NC_CACHE = None


def _get_nc():
    global _NC_CACHE
    if _NC_CACHE is None:
        _NC_CACHE = _build()
    return _NC_CACHE


def _shard_inputs(x, Wq, Wk, Wv, Wo, bo, W1, b1, W2, b2, g1, be1, g2, be2):
    """Build the 8 per-core input maps."""
    bf = lambda a: np.ascontiguousarray(a).astype(BF16NP)
    f32 = lambda a: np.ascontiguousarray(a, dtype=np.float32)

    in_maps = []
    for c in range(NCORES):
        b, half = divmod(c, TP)
        heads = slice(half * LH, (half + 1) * LH)
        ffs = slice(half * LFF, (half + 1) * LFF)
        hes = slice(half * LHE, (half + 1) * LHE)
        wq_l = np.concatenate(list(np.asarray(Wq)[heads]), axis=1)
        wk_l = np.concatenate(list(np.asarray(Wk)[heads]), axis=1)
        wv_l = np.concatenate(list(np.asarray(Wv)[heads]), axis=1)
        in_maps.append({
            "xbT": bf(np.asarray(x)[b].T),
            "wq": bf(wq_l), "wk": bf(wk_l), "wv": bf(wv_l),
            "wo": bf(np.asarray(Wo)[hes, :]),
            "w1": bf(np.asarray(W1)[:, ffs]), "w2": bf(np.asarray(W2)[ffs, :]),
            "b1l": f32(np.asarray(b1)[ffs]),
            "bo": f32(bo), "b2": f32(b2),
            "g1": f32(g1), "be1": f32(be1), "g2": f32(g2), "be2": f32(be2),
        })
    return in_maps


def kernel(x, Wq, Wk, Wv, Wo, bo, W1, b1, W2, b2, g1, be1, g2, be2,
           _trace=False):
    nc = _get_nc()
    in_maps = _shard_inputs(x, Wq, Wk, Wv, Wo, bo, W1, b1, W2, b2,
                            g1, be1, g2, be2)
    res = run_bass_kernel_spmd(nc, in_maps, list(range(NCORES)),
                               trace=_trace)
    out = np.empty((B, T, D), dtype=np.float32)
    for b in range(B):
        out[b] = res.results[TP * b]["out"].T
    if _trace:
        kernel.last_exec_time_ns = res.exec_time_ns
        kernel.last_results = res
    return out
